# revision 1
# baseline (speedup 1.0000x reference)
"""Trainium2 Bass kernel for nn_ExchangeableLayer (segment_reduce).

out[e] = relu( x[e] @ th00
             + (segmean(t0, cols) @ th10)[c_e]
             + (segmean(t0, rows) @ th01)[r_e]
             + (segmean(t1, t1cols) @ th1x0_10)[c_e]
             + (segmean(t2, t2rows) @ th2x0_01)[r_e]
             + mean(t0) @ th11 + mean(t1) @ th1x0_11 + mean(t2) @ th2x0_11
             + theta_b )

Strategy: sort entries by segment id on host, shard contiguously by segment
range across 8 cores.  Per core:
  A) segment sums via PE one-hot matmuls into per-128-segment PSUM windows
     (tables kept transposed [64, segs] in SBUF)
  B) scale by host-precomputed 1/(cnt+eps), apply thetas (PE), fold the
     global-mean term into the col table, transpose back to row-major,
     AllReduce (grand totals) + AllGather (final [seg, 64] bf16 tables)
  C) per-entry: relu(x @ th00 + ct[col] + rt[row]); x @ th00 uses
     host-pre-transposed bf16 x tiles with 2-way K=64 PE row packing;
     ct/rt rows fetched with batched indirect-DMA gathers.
"""

import math
import os
import sys
import types

import numpy as np

for _p in ("/root/.axon_site/_ro/trn_rl_repo", "/opt/trn_rl_repo"):
    if os.path.isdir(_p) and _p not in sys.path:
        sys.path.append(_p)

import ml_dtypes

import concourse.bass as bass
import concourse.mybir as mybir
from concourse import bacc, tile
from concourse.bass_utils import run_bass_kernel_spmd

BF16 = ml_dtypes.bfloat16
F32 = np.float32
NCORES = 8
U = 64
WIN = 128
EPS = 1e-10

# Full-size problem dims (the graded problem).
FULL_DIMS = dict(N=50000, M=10000, NNZ0=1_000_000, NNZ1=500_000, NNZ2=500_000)


# --------------------------------------------------------------------------
# host-side preparation
# --------------------------------------------------------------------------

def _prep_stream(ids, seg_sl):
    """Sort entries by id, shard contiguously at multiples of seg_sl.

    Returns stream dict with per-core window->tile assignments.
    """
    order = np.argsort(ids, kind="stable").astype(np.int64)
    sids = ids[order]
    bounds = np.searchsorted(sids, seg_sl * np.arange(NCORES + 1)).astype(np.int64)
    NW = -(-seg_sl // WIN)
    cores = []
    kmax = 1
    for c in range(NCORES):
        lo, hi = int(bounds[c]), int(bounds[c + 1])
        clen = hi - lo
        loc = (sids[lo:hi] - seg_sl * c).astype(np.int64)
        tc = -(-clen // 128)
        ws = np.searchsorted(loc, WIN * np.arange(NW + 1))
        wt = []
        for w in range(NW):
            a, b = int(ws[w]), int(ws[w + 1])
            if b > a:
                t0, t1 = a // 128, (b - 1) // 128
                wt.append((t0, t1 - t0 + 1))
                kmax = max(kmax, t1 - t0 + 1)
            else:
                wt.append((0, 0))
        cores.append(dict(clen=clen, loc=loc, corder=order[lo:hi], tc=tc, wt=wt))
    return dict(NW=NW, kmax=kmax, cores=cores)


def _mat_stream(stream, S, nnz):
    """Materialize per-core slot arrays: entry indices + rel ids."""
    NW, K = stream["NW"], stream["kmax"]
    for core in stream["cores"]:
        idx = np.full((S, 128), nnz, np.int64)
        rel = np.full((S, 128), -1.0, np.float32)
        tc, clen = core["tc"], core["clen"]
        locp = np.full(tc * 128, -(10 ** 6), np.int64)
        locp[:clen] = core["loc"]
        cordp = np.full(tc * 128, nnz, np.int64)
        cordp[:clen] = core["corder"]
        first_slot = np.full(max(tc, 1), -1, np.int64)
        for w, (t0, nt) in enumerate(core["wt"]):
            for k in range(nt):
                t = t0 + k
                s = w * K + k
                idx[s] = cordp[t * 128:(t + 1) * 128]
                rel[s] = locp[t * 128:(t + 1) * 128] - WIN * w
                if first_slot[t] < 0:
                    first_slot[t] = s
        core["idx"] = idx
        core["rel"] = rel
        core["first_slot"] = first_slot


def _prepare(inputs, dims):
    """All host-side metadata + per-core input arrays."""
    N, M = dims["N"], dims["M"]
    NNZ0, NNZ1, NNZ2 = dims["NNZ0"], dims["NNZ1"], dims["NNZ2"]
    M_SL, N_SL = M // NCORES, N // NCORES

    t0_rows = np.asarray(inputs["t0_rows"], np.int64)
    t0_cols = np.asarray(inputs["t0_cols"], np.int64)
    t1_cols = np.asarray(inputs["t1_cols"], np.int64)
    t2_rows = np.asarray(inputs["t2_rows"], np.int64)

    st0c = _prep_stream(t0_cols, M_SL)
    st0r = _prep_stream(t0_rows, N_SL)
    st1c = _prep_stream(t1_cols, M_SL)
    st2r = _prep_stream(t2_rows, N_SL)

    # uniform slot counts (pad S0c to a multiple of 64 for phase-C macros)
    S0c = -(-(st0c["NW"] * st0c["kmax"]) // 64) * 64
    S0r = st0r["NW"] * st0r["kmax"]
    S1c = st1c["NW"] * st1c["kmax"]
    S2r = st2r["NW"] * st2r["kmax"]

    _mat_stream(st0c, S0c, NNZ0)
    _mat_stream(st0r, S0r, NNZ0)
    _mat_stream(st1c, S1c, NNZ1)
    _mat_stream(st2r, S2r, NNZ2)

    NWc, NWr = st0c["NW"], st0r["NW"]
    MP, NP = NWc * 128, NWr * 128          # padded per-core table slice rows
    TBL = MP + NP                           # rows per core in gathered table

    x0 = np.asarray(inputs["t0_values"], np.float32)
    x1 = np.asarray(inputs["t1_values"], np.float32)
    x2 = np.asarray(inputs["t2_values"], np.float32)
    x0e = np.concatenate([x0, np.zeros((1, U), np.float32)]).astype(BF16)
    x1e = np.concatenate([x1, np.zeros((1, U), np.float32)]).astype(BF16)
    x2e = np.concatenate([x2, np.zeros((1, U), np.float32)]).astype(BF16)

    # inverse counts (global, then per-core padded slices)
    def _inv(ids, nseg):
        cnt = np.bincount(ids, minlength=nseg).astype(np.float32)
        return (1.0 / (cnt + np.float32(EPS))).astype(np.float32)

    inv_c0 = _inv(t0_cols, M)
    inv_r0 = _inv(t0_rows, N)
    inv_c1 = _inv(t1_cols, M)
    inv_r2 = _inv(t2_rows, N)

    def _slice_pad(arr, sl, pad_to):
        out = np.ones(pad_to, np.float32)
        out[: sl.stop - sl.start] = arr[sl]
        return out

    # gather positions
    cext = np.concatenate([t0_cols, [0]])
    rext = np.concatenate([t0_rows, [0]])

    # shared constants
    iota_b = np.broadcast_to(np.arange(128, dtype=np.float32), (128, 128)).astype(BF16)
    ident_f = np.eye(128, dtype=np.float32)
    ones_f = np.ones((1, U), np.float32)
    th = {k: np.asarray(inputs[k], np.float32) for k in
          ("theta_00", "theta_10", "theta_01", "theta_11", "theta_1x0_10",
           "theta_1x0_11", "theta_2x0_01", "theta_2x0_11")}
    th00_2 = np.concatenate([th["theta_00"], th["theta_00"]]).astype(BF16)  # [128, 64]
    thbT = np.asarray(inputs["theta_b"], np.float32).reshape(U, 1)

    in_maps = []
    post = []
    for c in range(NCORES):
        c0, r0, c1, r2 = (st0c["cores"][c], st0r["cores"][c],
                          st1c["cores"][c], st2r["cores"][c])
        x0c_a = x0e[c0["idx"]]                      # [S0c, 128, 64] bf16
        # phase-C transposed pairs: [128, (S0c//2)*128]
        xs = x0c_a.reshape(S0c // 2, 2, 128, U)
        xT2 = np.ascontiguousarray(
            xs.transpose(1, 3, 0, 2).reshape(128, (S0c // 2) * 128))

        cc = cext[c0["idx"]]                        # [S0c, 128]
        rr = rext[c0["idx"]]
        bias = 32767 if NCORES * TBL > 32767 else 0
        cpos = (TBL * (cc // M_SL) + (cc - M_SL * (cc // M_SL))
                - bias).astype(np.int16)
        rpos = (TBL * (rr // N_SL) + MP + (rr - N_SL * (rr // N_SL))
                - bias).astype(np.int16)

        def _wrap_idx(pos):
            # pos [S0c, 128] int16 -> [128, S0c*8+nm*8] in dma_gather layout:
            # per 64-slot macro, flat i = t*128+p lives at
            # (partition i%16, col i//16), replicated over 8 16-row groups.
            # 128 non-negative sentinel indices are appended per macro so the
            # gather ucode never sees trailing negatives (it drops those).
            nm = pos.shape[0] // 64
            blocks = pos.reshape(nm, 64 * 128)          # v[i] = pos[t, p]
            blocks = np.concatenate(
                [blocks, np.zeros((nm, 128), np.int16)], axis=1)
            w = blocks.reshape(nm, 520, 16).transpose(0, 2, 1)  # [nm, 16, 520]
            w = np.concatenate([w] * 8, axis=1)         # [nm, 128, 520]
            return np.ascontiguousarray(
                w.transpose(1, 0, 2).reshape(128, nm * 520))

        m = dict(
            x0c_a=x0c_a,
            x0r_a=x0e[r0["idx"]],
            x1c_a=x1e[c1["idx"]],
            x2r_a=x2e[r2["idx"]],
            xT2=xT2,
            rel0c=np.ascontiguousarray(c0["rel"].T).astype(BF16),
            rel0r=np.ascontiguousarray(r0["rel"].T).astype(BF16),
            rel1c=np.ascontiguousarray(c1["rel"].T).astype(BF16),
            rel2r=np.ascontiguousarray(r2["rel"].T).astype(BF16),
            cpos=_wrap_idx(cpos),                   # [128, S0c*8] int16
            rpos=_wrap_idx(rpos),
            inv_c0=_slice_pad(inv_c0, slice(c * M_SL, (c + 1) * M_SL), MP).reshape(1, MP),
            inv_r0=_slice_pad(inv_r0, slice(c * N_SL, (c + 1) * N_SL), NP).reshape(1, NP),
            inv_c1=_slice_pad(inv_c1, slice(c * M_SL, (c + 1) * M_SL), MP).reshape(1, MP),
            inv_r2=_slice_pad(inv_r2, slice(c * N_SL, (c + 1) * N_SL), NP).reshape(1, NP),
            iota_b=iota_b,
            ident_f=ident_f,
            ones_f=ones_f,
            th10=th["theta_10"], th1x0_10=th["theta_1x0_10"],
            th01=th["theta_01"], th2x0_01=th["theta_2x0_01"],
            th11=th["theta_11"], th1x0_11=th["theta_1x0_11"],
            th2x0_11=th["theta_2x0_11"],
            th00_2=th00_2,
            thbT=thbT,
        )
        in_maps.append(m)
        post.append(dict(first_slot=c0["first_slot"], clen=c0["clen"],
                         corder=c0["corder"]))

    meta = dict(
        S0c=S0c, S0r=S0r, S1c=S1c, S2r=S2r,
        K0c=st0c["kmax"], K0r=st0r["kmax"], K1c=st1c["kmax"], K2r=st2r["kmax"],
        NWc=NWc, NWr=NWr, MP=MP, NP=NP, TBL=TBL,
        NNZ0=NNZ0, NNZ1=NNZ1, NNZ2=NNZ2,
    )
    return meta, in_maps, post


# --------------------------------------------------------------------------
# device program
# --------------------------------------------------------------------------

_PROG_CACHE = {}


def _build_program(meta, debug=False):
    key = (tuple(sorted(meta.items())), debug)
    if key in _PROG_CACHE:
        return _PROG_CACHE[key]

    S0c, S0r, S1c, S2r = meta["S0c"], meta["S0r"], meta["S1c"], meta["S2r"]
    K0c, K0r, K1c, K2r = meta["K0c"], meta["K0r"], meta["K1c"], meta["K2r"]
    NWc, NWr = meta["NWc"], meta["NWr"]
    MP, NP, TBL = meta["MP"], meta["NP"], meta["TBL"]
    dt = mybir.dt
    AX = bass.mybir.AxisListType if hasattr(bass.mybir, "AxisListType") else None

    nc = bacc.Bacc("TRN2", target_bir_lowering=False, debug=False,
                   num_devices=NCORES)

    def din(name, shape, dty):
        return nc.dram_tensor(name, list(shape), dty, kind="ExternalInput")

    x0c_a = din("x0c_a", [S0c, 128, U], dt.bfloat16)
    x0r_a = din("x0r_a", [S0r, 128, U], dt.bfloat16)
    x1c_a = din("x1c_a", [S1c, 128, U], dt.bfloat16)
    x2r_a = din("x2r_a", [S2r, 128, U], dt.bfloat16)
    xT2 = din("xT2", [128, (S0c // 2) * 128], dt.bfloat16)
    rel0c = din("rel0c", [128, S0c], dt.bfloat16)
    rel0r = din("rel0r", [128, S0r], dt.bfloat16)
    rel1c = din("rel1c", [128, S1c], dt.bfloat16)
    rel2r = din("rel2r", [128, S2r], dt.bfloat16)
    cpos = din("cpos", [128, (S0c // 64) * 520], dt.int16)
    rpos = din("rpos", [128, (S0c // 64) * 520], dt.int16)
    inv_c0 = din("inv_c0", [1, MP], dt.float32)
    inv_r0 = din("inv_r0", [1, NP], dt.float32)
    inv_c1 = din("inv_c1", [1, MP], dt.float32)
    inv_r2 = din("inv_r2", [1, NP], dt.float32)
    iota_b = din("iota_b", [128, 128], dt.bfloat16)
    ident_f = din("ident_f", [128, 128], dt.float32)
    ones_f = din("ones_f", [1, U], dt.float32)
    th10 = din("th10", [U, U], dt.float32)
    th1x0_10 = din("th1x0_10", [U, U], dt.float32)
    th01 = din("th01", [U, U], dt.float32)
    th2x0_01 = din("th2x0_01", [U, U], dt.float32)
    th11 = din("th11", [U, U], dt.float32)
    th1x0_11 = din("th1x0_11", [U, U], dt.float32)
    th2x0_11 = din("th2x0_11", [U, U], dt.float32)
    th00_2 = din("th00_2", [128, U], dt.bfloat16)
    thbT = din("thbT", [U, 1], dt.float32)

    out_d = nc.dram_tensor("out_d", [S0c, 128, U], dt.float32,
                           kind="ExternalOutput")
    if debug:
        sum_dump = nc.dram_tensor("sum_dump", [U, 2 * (MP + NP)], dt.float32,
                                  kind="ExternalOutput")
        tbl_dump = nc.dram_tensor("tbl_dump", [NCORES * TBL, U], dt.float32,
                                  kind="ExternalOutput")
        ctg_dump = nc.dram_tensor("ctg_dump", [128, 64, U], dt.float32,
                                  kind="ExternalOutput")
        y0_dump = nc.dram_tensor("y0_dump", [64, 128, U], dt.float32,
                                 kind="ExternalOutput")

    TOT = 2 * (MP + NP)  # free-dim length of the transposed sums buffer
    off_c0, off_r0, off_c1, off_r2 = 0, MP, MP + NP, MP + NP + MP

    with tile.TileContext(nc) as tc:
        import contextlib
        with contextlib.ExitStack() as ctx:
            pp = ctx.enter_context(tc.tile_pool(name="persist", bufs=1))
            dram = ctx.enter_context(tc.tile_pool(name="dram", bufs=1, space="DRAM"))

            # SBUF freed after phase B (sums + inv rows are big)
            pab_cm = tc.tile_pool(name="pab", bufs=1)
            pab = pab_cm.__enter__()
            sumT = pab.tile([U, TOT], dt.float32)
            iota_t = pp.tile([128, 128], dt.bfloat16)
            nc.sync.dma_start(out=iota_t[:], in_=iota_b.ap())
            ident_t = pp.tile([128, 128], dt.float32)
            nc.sync.dma_start(out=ident_t[:], in_=ident_f.ap())
            ones_t = pp.tile([1, U], dt.float32)
            nc.sync.dma_start(out=ones_t[:], in_=ones_f.ap())
            ths = {}
            for nm, t in (("th10", th10), ("th1x0_10", th1x0_10), ("th01", th01),
                          ("th2x0_01", th2x0_01), ("th11", th11),
                          ("th1x0_11", th1x0_11), ("th2x0_11", th2x0_11)):
                ths[nm] = pp.tile([U, U], dt.float32, name=nm + "_t")
                nc.sync.dma_start(out=ths[nm][:], in_=t.ap())
            th00_t = pp.tile([128, U], dt.bfloat16)
            nc.sync.dma_start(out=th00_t[:], in_=th00_2.ap())
            thb_t = pp.tile([U, 1], dt.float32)
            nc.sync.dma_start(out=thb_t[:], in_=thbT.ap())

            # ---------------- phase A: windowed one-hot segment sums --------
            with tc.tile_pool(name="pa", bufs=3) as pa, \
                 tc.tile_pool(name="poh", bufs=8) as poh, \
                 tc.tile_pool(name="pas", bufs=2, space="PSUM") as pas, \
                 tc.tile_pool(name="prel", bufs=1) as prel:

                streams = [
                    (x0c_a, rel0c, K0c, NWc, off_c0, S0c),
                    (x0r_a, rel0r, K0r, NWr, off_r0, S0r),
                    (x1c_a, rel1c, K1c, NWc, off_c1, S1c),
                    (x2r_a, rel2r, K2r, NWr, off_r2, S2r),
                ]
                for si, (xa, rel_d, K, NW, soff, S) in enumerate(streams):
                    rel_t = prel.tile([128, S], dt.bfloat16, name=f"rel_t{si}",
                                      tag=f"rel{si}")
                    nc.sync.dma_start(out=rel_t[:], in_=rel_d.ap())
                    for w in range(NW):
                        xw = pa.tile([128, K, U], dt.bfloat16, tag="xw")
                        nc.sync.dma_start(
                            out=xw[:, :K, :],
                            in_=xa.ap()[w * K:(w + 1) * K].rearrange("s p f -> p s f"))
                        pw = pas.tile([U, 128], dt.float32, space="PSUM", tag="pw")
                        for k in range(K):
                            s = w * K + k
                            oh = poh.tile([128, 128], dt.bfloat16, tag="oh")
                            nc.vector.tensor_tensor(
                                out=oh[:],
                                in0=rel_t[:, s:s + 1].to_broadcast([128, 128]),
                                in1=iota_t[:],
                                op=mybir.AluOpType.is_equal)
                            nc.tensor.matmul(pw[:], lhsT=xw[:, k, :], rhs=oh[:],
                                             start=(k == 0), stop=(k == K - 1))
                        nc.vector.tensor_copy(
                            out=sumT[:, soff + w * 128: soff + (w + 1) * 128],
                            in_=pw[:])

            # ---------------- phase B: tables -------------------------------
            with tc.tile_pool(name="pb", bufs=2) as pb, \
                 tc.tile_pool(name="pbs", bufs=1, space="PSUM") as pbs:

                # grand totals (transposed): [64, 4] cols = t0, t1, t2
                totL = pp.tile([U, 4], dt.float32)
                nc.vector.memset(totL[:], 0.0)
                nc.vector.tensor_reduce(
                    out=totL[:, 0:1], in_=sumT[:, off_c0:off_c0 + MP],
                    axis=mybir.AxisListType.X, op=mybir.AluOpType.add)
                nc.vector.tensor_reduce(
                    out=totL[:, 1:2], in_=sumT[:, off_c1:off_c1 + MP],
                    axis=mybir.AxisListType.X, op=mybir.AluOpType.add)
                nc.vector.tensor_reduce(
                    out=totL[:, 2:3], in_=sumT[:, off_r2:off_r2 + NP],
                    axis=mybir.AxisListType.X, op=mybir.AluOpType.add)

                totb = dram.tile([U, 4], dt.float32)
                totg = dram.tile([U, 4], dt.float32, addr_space="Shared")
                nc.gpsimd.dma_start(out=totb[:], in_=totL[:])
                nc.gpsimd.collective_compute(
                    "AllReduce", mybir.AluOpType.add,
                    ins=[totb.opt()], outs=[totg.opt()],
                    replica_groups=[list(range(NCORES))])
                totG = pp.tile([U, 4], dt.float32)
                nc.gpsimd.dma_start(out=totG[:], in_=totg[:])

                mv = pp.tile([U, 4], dt.float32)
                for j, nnz in ((0, meta["NNZ0"]), (1, meta["NNZ1"]),
                               (2, meta["NNZ2"])):
                    nc.vector.tensor_scalar_mul(
                        out=mv[:, j:j + 1], in0=totG[:, j:j + 1],
                        scalar1=float(1.0 / nnz))
                gp = pbs.tile([U, 1], dt.float32, space="PSUM", tag="gp")
                nc.tensor.matmul(gp[:], lhsT=ths["th11"][:], rhs=mv[:, 0:1],
                                 start=True, stop=False)
                nc.tensor.matmul(gp[:], lhsT=ths["th1x0_11"][:], rhs=mv[:, 1:2],
                                 start=False, stop=False)
                nc.tensor.matmul(gp[:], lhsT=ths["th2x0_11"][:], rhs=mv[:, 2:3],
                                 start=False, stop=True)
                g_t = pp.tile([U, 1], dt.float32)
                nc.vector.tensor_add(out=g_t[:], in0=gp[:], in1=thb_t[:])

                invs = {}
                for nm, t, ln in (("inv_c0", inv_c0, MP), ("inv_r0", inv_r0, NP),
                                  ("inv_c1", inv_c1, MP), ("inv_r2", inv_r2, NP)):
                    invs[nm] = pab.tile([1, ln], dt.float32, name=nm + "_t")
                    nc.sync.dma_start(out=invs[nm][:], in_=t.ap())

                ctrt_slice = dram.tile([TBL, U], dt.float32)
                ctrt_all = dram.tile([NCORES * TBL, U], dt.float32,
                                     addr_space="Shared")

                ct_stage = pp.tile([128, NWc, U], dt.float32)
                rt_stage = pp.tile([128, NWr, U], dt.float32)

                def table_chunk(ci, inv_a, inv_b, soff_a, soff_b, thA, thB,
                                add_g, stage):
                    sl = slice(ci * 128, (ci + 1) * 128)
                    pr = pbs.tile([U, 128], dt.float32, space="PSUM", tag="pr")
                    nc.tensor.matmul(pr[:], lhsT=ones_t[:], rhs=inv_a[:, sl],
                                     start=True, stop=True)
                    m0 = pb.tile([U, 128], dt.float32, tag="m0")
                    nc.vector.tensor_mul(out=m0[:],
                                         in0=sumT[:, soff_a + ci * 128:
                                                  soff_a + (ci + 1) * 128],
                                         in1=pr[:])
                    pr2 = pbs.tile([U, 128], dt.float32, space="PSUM", tag="pr2")
                    nc.tensor.matmul(pr2[:], lhsT=ones_t[:], rhs=inv_b[:, sl],
                                     start=True, stop=True)
                    m1 = pb.tile([U, 128], dt.float32, tag="m1")
                    nc.vector.tensor_mul(out=m1[:],
                                         in0=sumT[:, soff_b + ci * 128:
                                                  soff_b + (ci + 1) * 128],
                                         in1=pr2[:])
                    pc = pbs.tile([U, 128], dt.float32, space="PSUM", tag="pc")
                    nc.tensor.matmul(pc[:], lhsT=thA[:], rhs=m0[:],
                                     start=True, stop=False)
                    nc.tensor.matmul(pc[:], lhsT=thB[:], rhs=m1[:],
                                     start=False, stop=True)
                    cf = pb.tile([U, 128], dt.float32, tag="cf")
                    if add_g:
                        nc.vector.tensor_tensor(
                            out=cf[:], in0=pc[:],
                            in1=g_t[:].to_broadcast([U, 128]),
                            op=mybir.AluOpType.add)
                    else:
                        nc.vector.tensor_copy(out=cf[:], in_=pc[:])
                    pt = pbs.tile([128, U], dt.float32, space="PSUM", tag="pt")
                    nc.tensor.transpose(out=pt[:], in_=cf[:],
                                        identity=ident_t[:U, :U])
                    nc.vector.tensor_copy(out=stage[:, ci, :], in_=pt[:])

                for ci in range(NWc):
                    table_chunk(ci, invs["inv_c0"], invs["inv_c1"], off_c0,
                                off_c1, ths["th10"], ths["th1x0_10"], True,
                                ct_stage)
                for ci in range(NWr):
                    table_chunk(ci, invs["inv_r0"], invs["inv_r2"], off_r0,
                                off_r2, ths["th01"], ths["th2x0_01"], False,
                                rt_stage)

                nc.sync.dma_start(
                    out=ctrt_slice[0:MP].rearrange("(c p) f -> p c f", p=128),
                    in_=ct_stage[:])
                nc.sync.dma_start(
                    out=ctrt_slice[MP:TBL].rearrange("(c p) f -> p c f", p=128),
                    in_=rt_stage[:])
                nc.gpsimd.collective_compute(
                    "AllGather", mybir.AluOpType.bypass,
                    ins=[ctrt_slice.opt()], outs=[ctrt_all.opt()],
                    replica_groups=[list(range(NCORES))])
                if debug:
                    nc.sync.dma_start(out=sum_dump.ap(), in_=sumT[:])
                    with tc.tile_pool(name="pdbg", bufs=2) as pdbg:
                        for b in range(NCORES * TBL // 128):
                            dtile = pdbg.tile([128, U], dt.float32, tag="dt")
                            nc.sync.dma_start(
                                out=dtile[:],
                                in_=ctrt_all[b * 128:(b + 1) * 128])
                            nc.sync.dma_start(
                                out=tbl_dump.ap()[b * 128:(b + 1) * 128],
                                in_=dtile[:])

            pab_cm.__exit__(None, None, None)

            # ---------------- phase C: per-entry output ---------------------
            with tc.tile_pool(name="pc1", bufs=2) as pc1, \
                 tc.tile_pool(name="pct", bufs=4) as pct, \
                 tc.tile_pool(name="pcs", bufs=6, space="PSUM") as pcs, \
                 tc.tile_pool(name="ppos", bufs=1) as ppos:

                cpos_t = ppos.tile([128, (S0c // 64) * 520], dt.int16)
                nc.sync.dma_start(out=cpos_t[:], in_=cpos.ap())
                rpos_t = ppos.tile([128, (S0c // 64) * 520], dt.int16)
                nc.sync.dma_start(out=rpos_t[:], in_=rpos.ap())

                bias_rows = 32767 if NCORES * TBL > 32767 else 0
                gather_src = ctrt_all[bias_rows:]

                NMAC = S0c // 64
                for m in range(NMAC):
                    xw2 = pc1.tile([128, 32 * 128], dt.bfloat16, tag="xw2")
                    nc.sync.dma_start(
                        out=xw2[:],
                        in_=xT2.ap()[:, m * 4096:(m + 1) * 4096])
                    ctg = pc1.tile([128, 65, U], dt.float32, tag="ctg")
                    nc.gpsimd.dma_gather(
                        out_ap=ctg[:], in_ap=gather_src,
                        idxs_ap=cpos_t[:, m * 520:(m + 1) * 520],
                        num_idxs=65 * 128, num_idxs_reg=65 * 128, elem_size=U,
                        single_packet=False)
                    rtg = pc1.tile([128, 65, U], dt.float32, tag="rtg")
                    nc.gpsimd.dma_gather(
                        out_ap=rtg[:], in_ap=gather_src,
                        idxs_ap=rpos_t[:, m * 520:(m + 1) * 520],
                        num_idxs=65 * 128, num_idxs_reg=65 * 128, elem_size=U,
                        single_packet=False)
                    if debug and m == 0:
                        nc.sync.dma_start(out=ctg_dump.ap(), in_=ctg[:])
                    ost = pc1.tile([128, 64, U], dt.float32, tag="ost")
                    for t in range(64):
                        q, j = t % 2, t // 2
                        py = pcs.tile([128, U], dt.float32, space="PSUM", tag="py")
                        nc.tensor.matmul(
                            py[:],
                            lhsT=xw2[64 * q:64 * (q + 1), j * 128:(j + 1) * 128],
                            rhs=th00_t[64 * q:64 * (q + 1), :],
                            start=True, stop=True)
                        if debug and m == 0:
                            yd = pct.tile([128, U], dt.float32, tag="yd")
                            nc.vector.tensor_copy(out=yd[:], in_=py[:])
                            nc.sync.dma_start(out=y0_dump.ap()[t], in_=yd[:])
                        t1 = pct.tile([128, U], dt.float32, tag="t1")
                        nc.vector.tensor_tensor(out=t1[:], in0=py[:],
                                                in1=ctg[:, t, :],
                                                op=mybir.AluOpType.add)
                        t2 = pct.tile([128, U], dt.float32, tag="t2")
                        nc.vector.tensor_tensor(out=t2[:], in0=t1[:],
                                                in1=rtg[:, t, :],
                                                op=mybir.AluOpType.add)
                        nc.scalar.activation(
                            out=ost[:, t, :], in_=t2[:],
                            func=mybir.ActivationFunctionType.Relu)
                    nc.sync.dma_start(
                        out=out_d.ap()[m * 64:(m + 1) * 64].rearrange(
                            "s p f -> p s f"),
                        in_=ost[:])

    nc.compile()
    _PROG_CACHE[key] = nc
    return nc


# --------------------------------------------------------------------------
# entry point
# --------------------------------------------------------------------------

def _run(inputs, dims, trace=False, debug=False):
    meta, in_maps, post = _prepare(inputs, dims)
    nc = _build_program(meta, debug=debug)
    res = run_bass_kernel_spmd(nc, in_maps, core_ids=list(range(NCORES)),
                               trace=trace)
    NNZ0 = dims["NNZ0"]
    out = np.empty((NNZ0, U), np.float32)
    for c in range(NCORES):
        o = res.results[c]["out_d"].reshape(-1, 128, U)
        p = post[c]
        if p["clen"] == 0:
            continue
        rows = o[p["first_slot"]].reshape(-1, U)[:p["clen"]]
        out[p["corder"]] = rows
    return out, res


def kernel(**inputs):
    out, _ = _run(inputs, FULL_DIMS, trace=False)
    return out


# ------- helpers for test harness ------------------------------------------

def install_ntff_hook():
    """Enable NTFF profiling under axon (exec_time_ns in results)."""
    try:
        import antenv
        import contextlib as _cl
        mod = types.ModuleType("antenv.axon_hooks")
        _h = [None]
        mod.set_axon_ntff_profile_hook = lambda h: _h.__setitem__(0, h)
        mod.get_axon_ntff_profile_hook = lambda: _h[0]
        sys.modules["antenv.axon_hooks"] = mod
        antenv.axon_hooks = mod
        from trn_agent_boot.trn_boot import _ntff_profile_via_ctypes
        mod.set_axon_ntff_profile_hook(
            _ntff_profile_via_ctypes("/opt/axon/libaxon_pjrt.so"))
        return True
    except Exception as e:  # pragma: no cover
        print("ntff hook install failed:", e)
        return False


def ref_numpy(inputs, dims):
    """Numpy port of the reference (for arbitrary dims)."""
    N, M = dims["N"], dims["M"]
    x0 = np.asarray(inputs["t0_values"], np.float64)
    x1 = np.asarray(inputs["t1_values"], np.float64)
    x2 = np.asarray(inputs["t2_values"], np.float64)
    tr = np.asarray(inputs["t0_rows"]); tcl = np.asarray(inputs["t0_cols"])
    t1c = np.asarray(inputs["t1_cols"]); t2r = np.asarray(inputs["t2_rows"])

    def segmean(v, ids, n):
        s = np.zeros((n, v.shape[1])); np.add.at(s, ids, v)
        c = np.bincount(ids, minlength=n).astype(np.float64)
        return s / (c + EPS)[:, None]

    th = {k: np.asarray(inputs[k], np.float64) for k in
          ("theta_00", "theta_10", "theta_01", "theta_11", "theta_1x0_10",
           "theta_1x0_11", "theta_2x0_01", "theta_2x0_11")}
    vals = x0 @ th["theta_00"]
    vals += (segmean(x0, tcl, M) @ th["theta_10"])[tcl]
    vals += (segmean(x0, tr, N) @ th["theta_01"])[tr]
    vals += x0.mean(0) @ th["theta_11"]
    vals += (segmean(x1, t1c, M) @ th["theta_1x0_10"])[tcl]
    vals += x1.mean(0) @ th["theta_1x0_11"]
    vals += (segmean(x2, t2r, N) @ th["theta_2x0_01"])[tr]
    vals += x2.mean(0) @ th["theta_2x0_11"]
    vals += np.asarray(inputs["theta_b"], np.float64)
    return np.maximum(vals, 0.0).astype(np.float32)



# revision 3
# speedup vs baseline: 2.7943x; 2.7943x over previous
"""Trainium2 Bass kernel for nn_ExchangeableLayer (segment_reduce).

out[e] = relu( x[e] @ th00
             + (segmean(t0, cols) @ th10)[c_e]
             + (segmean(t0, rows) @ th01)[r_e]
             + (segmean(t1, t1cols) @ th1x0_10)[c_e]
             + (segmean(t2, t2rows) @ th2x0_01)[r_e]
             + mean(t0) @ th11 + mean(t1) @ th1x0_11 + mean(t2) @ th2x0_11
             + theta_b )

Strategy: sort entries by segment id on host, shard contiguously by segment
range across 8 cores.  Per core:
  A) segment sums via PE one-hot matmuls into per-128-segment PSUM windows
     (tables kept transposed [64, segs] in SBUF)
  B) scale by host-precomputed 1/(cnt+eps), apply thetas (PE), fold the
     global-mean term into the col table, transpose back to row-major,
     AllReduce (grand totals) + AllGather (final [seg, 64] bf16 tables)
  C) per-entry: relu(x @ th00 + ct[col] + rt[row]); x @ th00 uses
     host-pre-transposed bf16 x tiles with 2-way K=64 PE row packing;
     ct/rt rows fetched with batched indirect-DMA gathers.
"""

import math
import os
import sys
import types

import numpy as np

for _p in ("/root/.axon_site/_ro/trn_rl_repo", "/opt/trn_rl_repo"):
    if os.path.isdir(_p) and _p not in sys.path:
        sys.path.append(_p)

import ml_dtypes

import concourse.bass as bass
import concourse.mybir as mybir
from concourse import bacc, tile
from concourse.bass_utils import run_bass_kernel_spmd

BF16 = ml_dtypes.bfloat16
F32 = np.float32
NCORES = 8
U = 64
WIN = 128
EPS = 1e-10

# Full-size problem dims (the graded problem).
FULL_DIMS = dict(N=50000, M=10000, NNZ0=1_000_000, NNZ1=500_000, NNZ2=500_000)


# --------------------------------------------------------------------------
# host-side preparation
# --------------------------------------------------------------------------

def _prep_stream(ids, seg_sl):
    """Sort entries by id, shard contiguously at multiples of seg_sl.

    Returns stream dict with per-core window->tile assignments.
    """
    order = np.argsort(ids, kind="stable").astype(np.int64)
    sids = ids[order]
    bounds = np.searchsorted(sids, seg_sl * np.arange(NCORES + 1)).astype(np.int64)
    NW = -(-seg_sl // WIN)
    cores = []
    kmax = 1
    for c in range(NCORES):
        lo, hi = int(bounds[c]), int(bounds[c + 1])
        clen = hi - lo
        loc = (sids[lo:hi] - seg_sl * c).astype(np.int64)
        tc = -(-clen // 128)
        ws = np.searchsorted(loc, WIN * np.arange(NW + 1))
        wt = []
        for w in range(NW):
            a, b = int(ws[w]), int(ws[w + 1])
            if b > a:
                t0, t1 = a // 128, (b - 1) // 128
                wt.append((t0, t1 - t0 + 1))
                kmax = max(kmax, t1 - t0 + 1)
            else:
                wt.append((0, 0))
        cores.append(dict(clen=clen, loc=loc, corder=order[lo:hi], tc=tc, wt=wt))
    return dict(NW=NW, kmax=kmax, cores=cores)


def _mat_stream(stream, S, nnz):
    """Materialize per-core slot arrays: entry indices + rel ids."""
    NW, K = stream["NW"], stream["kmax"]
    for core in stream["cores"]:
        idx = np.full((S, 128), nnz, np.int64)
        rel = np.full((S, 128), -1.0, np.float32)
        tc, clen = core["tc"], core["clen"]
        locp = np.full(tc * 128, -(10 ** 6), np.int64)
        locp[:clen] = core["loc"]
        cordp = np.full(tc * 128, nnz, np.int64)
        cordp[:clen] = core["corder"]
        first_slot = np.full(max(tc, 1), -1, np.int64)
        for w, (t0, nt) in enumerate(core["wt"]):
            for k in range(nt):
                t = t0 + k
                s = w * K + k
                idx[s] = cordp[t * 128:(t + 1) * 128]
                rel[s] = locp[t * 128:(t + 1) * 128] - WIN * w
                if first_slot[t] < 0:
                    first_slot[t] = s
        core["idx"] = idx
        core["rel"] = rel
        core["first_slot"] = first_slot


def _prepare(inputs, dims):
    """All host-side metadata + per-core input arrays."""
    N, M = dims["N"], dims["M"]
    NNZ0, NNZ1, NNZ2 = dims["NNZ0"], dims["NNZ1"], dims["NNZ2"]
    M_SL, N_SL = M // NCORES, N // NCORES

    t0_rows = np.asarray(inputs["t0_rows"], np.int64)
    t0_cols = np.asarray(inputs["t0_cols"], np.int64)
    t1_cols = np.asarray(inputs["t1_cols"], np.int64)
    t2_rows = np.asarray(inputs["t2_rows"], np.int64)

    st0c = _prep_stream(t0_cols, M_SL)
    st0r = _prep_stream(t0_rows, N_SL)
    st1c = _prep_stream(t1_cols, M_SL)
    st2r = _prep_stream(t2_rows, N_SL)

    # uniform slot counts (pad S0c to a multiple of 64 for phase-C macros)
    S0c = -(-(st0c["NW"] * st0c["kmax"]) // 64) * 64
    S0r = st0r["NW"] * st0r["kmax"]
    S1c = st1c["NW"] * st1c["kmax"]
    S2r = st2r["NW"] * st2r["kmax"]

    _mat_stream(st0c, S0c, NNZ0)
    _mat_stream(st0r, S0r, NNZ0)
    _mat_stream(st1c, S1c, NNZ1)
    _mat_stream(st2r, S2r, NNZ2)

    NWc, NWr = st0c["NW"], st0r["NW"]
    MP, NP = NWc * 128, NWr * 128          # padded per-core table slice rows
    TBL = MP + NP                           # rows per core in gathered table

    x0 = np.asarray(inputs["t0_values"], np.float32)
    x1 = np.asarray(inputs["t1_values"], np.float32)
    x2 = np.asarray(inputs["t2_values"], np.float32)
    x0e = np.concatenate([x0, np.zeros((1, U), np.float32)]).astype(BF16)
    x1e = np.concatenate([x1, np.zeros((1, U), np.float32)]).astype(BF16)
    x2e = np.concatenate([x2, np.zeros((1, U), np.float32)]).astype(BF16)

    # inverse counts (global, then per-core padded slices)
    def _inv(ids, nseg):
        cnt = np.bincount(ids, minlength=nseg).astype(np.float32)
        return (1.0 / (cnt + np.float32(EPS))).astype(np.float32)

    inv_c0 = _inv(t0_cols, M)
    inv_r0 = _inv(t0_rows, N)
    inv_c1 = _inv(t1_cols, M)
    inv_r2 = _inv(t2_rows, N)

    def _slice_pad(arr, sl, pad_to):
        out = np.ones(pad_to, np.float32)
        out[: sl.stop - sl.start] = arr[sl]
        return out

    # gather positions
    cext = np.concatenate([t0_cols, [0]])
    rext = np.concatenate([t0_rows, [0]])

    # shared constants
    iota_b = np.broadcast_to(np.arange(128, dtype=np.float32), (128, 128)).astype(BF16)
    ident_f = np.eye(128, dtype=np.float32)
    ones_f = np.ones((1, U), np.float32)
    th = {k: np.asarray(inputs[k], np.float32) for k in
          ("theta_00", "theta_10", "theta_01", "theta_11", "theta_1x0_10",
           "theta_1x0_11", "theta_2x0_01", "theta_2x0_11")}
    th00_2 = np.concatenate([th["theta_00"], th["theta_00"]]).astype(BF16)  # [128, 64]
    thbT = np.asarray(inputs["theta_b"], np.float32).reshape(U, 1)

    in_maps = []
    post = []
    for c in range(NCORES):
        c0, r0, c1, r2 = (st0c["cores"][c], st0r["cores"][c],
                          st1c["cores"][c], st2r["cores"][c])
        x0c_a = x0e[c0["idx"]]                      # [S0c, 128, 64] bf16
        # phase-C transposed pairs: [128, (S0c//2)*128]
        xs = x0c_a.reshape(S0c // 2, 2, 128, U)
        xT2 = np.ascontiguousarray(
            xs.transpose(1, 3, 0, 2).reshape(128, (S0c // 2) * 128))

        cc = cext[c0["idx"]]                        # [S0c, 128]
        rr = rext[c0["idx"]]
        bias = 32767 if NCORES * TBL > 32767 else 0
        cpos = (TBL * (cc // M_SL) + (cc - M_SL * (cc // M_SL))
                - bias).astype(np.int16)
        rpos = (TBL * (rr // N_SL) + MP + (rr - N_SL * (rr // N_SL))
                - bias).astype(np.int16)

        def _wrap_idx(pos):
            # pos [S0c, 128] int16 -> [128, S0c*8+nm*8] in dma_gather layout:
            # per 64-slot macro, flat i = t*128+p lives at
            # (partition i%16, col i//16), replicated over 8 16-row groups.
            # 128 non-negative sentinel indices are appended per macro so the
            # gather ucode never sees trailing negatives (it drops those).
            nm = pos.shape[0] // 64
            blocks = pos.reshape(nm, 64 * 128)          # v[i] = pos[t, p]
            blocks = np.concatenate(
                [blocks, np.zeros((nm, 128), np.int16)], axis=1)
            w = blocks.reshape(nm, 520, 16).transpose(0, 2, 1)  # [nm, 16, 520]
            w = np.concatenate([w] * 8, axis=1)         # [nm, 128, 520]
            return np.ascontiguousarray(
                w.transpose(1, 0, 2).reshape(128, nm * 520))

        m = dict(
            x0c_a=x0c_a,
            x0r_a=x0e[r0["idx"]],
            x1c_a=x1e[c1["idx"]],
            x2r_a=x2e[r2["idx"]],
            xT2=xT2,
            rel0c=np.ascontiguousarray(c0["rel"].T).astype(BF16),
            rel0r=np.ascontiguousarray(r0["rel"].T).astype(BF16),
            rel1c=np.ascontiguousarray(c1["rel"].T).astype(BF16),
            rel2r=np.ascontiguousarray(r2["rel"].T).astype(BF16),
            cpos=_wrap_idx(cpos),                   # [128, S0c*8] int16
            rpos=_wrap_idx(rpos),
            inv_c0=_slice_pad(inv_c0, slice(c * M_SL, (c + 1) * M_SL), MP).reshape(1, MP),
            inv_r0=_slice_pad(inv_r0, slice(c * N_SL, (c + 1) * N_SL), NP).reshape(1, NP),
            inv_c1=_slice_pad(inv_c1, slice(c * M_SL, (c + 1) * M_SL), MP).reshape(1, MP),
            inv_r2=_slice_pad(inv_r2, slice(c * N_SL, (c + 1) * N_SL), NP).reshape(1, NP),
            iota_b=iota_b,
            ident_f=ident_f,
            ones_f=ones_f,
            th10=th["theta_10"], th1x0_10=th["theta_1x0_10"],
            th01=th["theta_01"], th2x0_01=th["theta_2x0_01"],
            th11=th["theta_11"], th1x0_11=th["theta_1x0_11"],
            th2x0_11=th["theta_2x0_11"],
            th00_2=th00_2,
            thbT=thbT,
        )
        in_maps.append(m)
        post.append(dict(first_slot=c0["first_slot"], clen=c0["clen"],
                         corder=c0["corder"]))

    meta = dict(
        S0c=S0c, S0r=S0r, S1c=S1c, S2r=S2r,
        K0c=st0c["kmax"], K0r=st0r["kmax"], K1c=st1c["kmax"], K2r=st2r["kmax"],
        NWc=NWc, NWr=NWr, MP=MP, NP=NP, TBL=TBL,
        NNZ0=NNZ0, NNZ1=NNZ1, NNZ2=NNZ2,
    )
    return meta, in_maps, post


# --------------------------------------------------------------------------
# device program
# --------------------------------------------------------------------------

_PROG_CACHE = {}


def _build_program(meta, debug=False):
    key = (tuple(sorted(meta.items())), debug)
    if key in _PROG_CACHE:
        return _PROG_CACHE[key]

    S0c, S0r, S1c, S2r = meta["S0c"], meta["S0r"], meta["S1c"], meta["S2r"]
    K0c, K0r, K1c, K2r = meta["K0c"], meta["K0r"], meta["K1c"], meta["K2r"]
    NWc, NWr = meta["NWc"], meta["NWr"]
    MP, NP, TBL = meta["MP"], meta["NP"], meta["TBL"]
    dt = mybir.dt
    AX = bass.mybir.AxisListType if hasattr(bass.mybir, "AxisListType") else None

    nc = bacc.Bacc("TRN2", target_bir_lowering=False, debug=False,
                   num_devices=NCORES, num_swdge_queues=4)

    def din(name, shape, dty):
        return nc.dram_tensor(name, list(shape), dty, kind="ExternalInput")

    x0c_a = din("x0c_a", [S0c, 128, U], dt.bfloat16)
    x0r_a = din("x0r_a", [S0r, 128, U], dt.bfloat16)
    x1c_a = din("x1c_a", [S1c, 128, U], dt.bfloat16)
    x2r_a = din("x2r_a", [S2r, 128, U], dt.bfloat16)
    xT2 = din("xT2", [128, (S0c // 2) * 128], dt.bfloat16)
    rel0c = din("rel0c", [128, S0c], dt.bfloat16)
    rel0r = din("rel0r", [128, S0r], dt.bfloat16)
    rel1c = din("rel1c", [128, S1c], dt.bfloat16)
    rel2r = din("rel2r", [128, S2r], dt.bfloat16)
    cpos = din("cpos", [128, (S0c // 64) * 520], dt.int16)
    rpos = din("rpos", [128, (S0c // 64) * 520], dt.int16)
    inv_c0 = din("inv_c0", [1, MP], dt.float32)
    inv_r0 = din("inv_r0", [1, NP], dt.float32)
    inv_c1 = din("inv_c1", [1, MP], dt.float32)
    inv_r2 = din("inv_r2", [1, NP], dt.float32)
    iota_b = din("iota_b", [128, 128], dt.bfloat16)
    ident_f = din("ident_f", [128, 128], dt.float32)
    ones_f = din("ones_f", [1, U], dt.float32)
    th10 = din("th10", [U, U], dt.float32)
    th1x0_10 = din("th1x0_10", [U, U], dt.float32)
    th01 = din("th01", [U, U], dt.float32)
    th2x0_01 = din("th2x0_01", [U, U], dt.float32)
    th11 = din("th11", [U, U], dt.float32)
    th1x0_11 = din("th1x0_11", [U, U], dt.float32)
    th2x0_11 = din("th2x0_11", [U, U], dt.float32)
    th00_2 = din("th00_2", [128, U], dt.bfloat16)
    thbT = din("thbT", [U, 1], dt.float32)

    out_d = nc.dram_tensor("out_d", [S0c, 128, U], dt.float32,
                           kind="ExternalOutput")
    if debug:
        sum_dump = nc.dram_tensor("sum_dump", [U, 2 * (MP + NP)], dt.float32,
                                  kind="ExternalOutput")
        tbl_dump = nc.dram_tensor("tbl_dump", [NCORES * TBL, U], dt.float32,
                                  kind="ExternalOutput")
        ctg_dump = nc.dram_tensor("ctg_dump", [128, 64, U], dt.float32,
                                  kind="ExternalOutput")
        y0_dump = nc.dram_tensor("y0_dump", [64, 128, U], dt.float32,
                                 kind="ExternalOutput")

    TOT = 2 * (MP + NP)  # free-dim length of the transposed sums buffer
    off_c0, off_r0, off_c1, off_r2 = 0, MP, MP + NP, MP + NP + MP

    with tile.TileContext(nc) as tc:
        import contextlib
        with contextlib.ExitStack() as ctx:
            pp = ctx.enter_context(tc.tile_pool(name="persist", bufs=1))
            dram = ctx.enter_context(tc.tile_pool(name="dram", bufs=1, space="DRAM"))

            # SBUF freed after phase B (sums + inv rows are big)
            pab_cm = tc.tile_pool(name="pab", bufs=1)
            pab = pab_cm.__enter__()
            sumT = pab.tile([U, TOT], dt.float32)
            iota_t = pp.tile([128, 128], dt.bfloat16)
            nc.sync.dma_start(out=iota_t[:], in_=iota_b.ap())
            ident_t = pp.tile([128, 128], dt.float32)
            nc.sync.dma_start(out=ident_t[:], in_=ident_f.ap())
            ones_t = pp.tile([1, U], dt.float32)
            nc.sync.dma_start(out=ones_t[:], in_=ones_f.ap())
            ths = {}
            for nm, t in (("th10", th10), ("th1x0_10", th1x0_10), ("th01", th01),
                          ("th2x0_01", th2x0_01), ("th11", th11),
                          ("th1x0_11", th1x0_11), ("th2x0_11", th2x0_11)):
                ths[nm] = pp.tile([U, U], dt.float32, name=nm + "_t")
                nc.sync.dma_start(out=ths[nm][:], in_=t.ap())
            th00_t = pp.tile([128, U], dt.bfloat16)
            nc.sync.dma_start(out=th00_t[:], in_=th00_2.ap())
            thb_t = pp.tile([U, 1], dt.float32)
            nc.sync.dma_start(out=thb_t[:], in_=thbT.ap())

            # ---------------- phase A: windowed one-hot segment sums --------
            with tc.tile_pool(name="pa", bufs=3) as pa, \
                 tc.tile_pool(name="poh", bufs=8) as poh, \
                 tc.tile_pool(name="pas", bufs=2, space="PSUM") as pas, \
                 tc.tile_pool(name="prel", bufs=1) as prel:

                streams = [
                    (x0c_a, rel0c, K0c, NWc, off_c0, S0c),
                    (x0r_a, rel0r, K0r, NWr, off_r0, S0r),
                    (x1c_a, rel1c, K1c, NWc, off_c1, S1c),
                    (x2r_a, rel2r, K2r, NWr, off_r2, S2r),
                ]
                for si, (xa, rel_d, K, NW, soff, S) in enumerate(streams):
                    rel_t = prel.tile([128, S], dt.bfloat16, name=f"rel_t{si}",
                                      tag=f"rel{si}")
                    nc.sync.dma_start(out=rel_t[:], in_=rel_d.ap())
                    for w in range(NW):
                        xw = pa.tile([128, K, U], dt.bfloat16, tag="xw")
                        nc.sync.dma_start(
                            out=xw[:, :K, :],
                            in_=xa.ap()[w * K:(w + 1) * K].rearrange("s p f -> p s f"))
                        pw = pas.tile([U, 128], dt.float32, space="PSUM", tag="pw")
                        for k in range(K):
                            s = w * K + k
                            oh = poh.tile([128, 128], dt.bfloat16, tag="oh")
                            nc.vector.tensor_tensor(
                                out=oh[:],
                                in0=rel_t[:, s:s + 1].to_broadcast([128, 128]),
                                in1=iota_t[:],
                                op=mybir.AluOpType.is_equal)
                            nc.tensor.matmul(pw[:], lhsT=xw[:, k, :], rhs=oh[:],
                                             start=(k == 0), stop=(k == K - 1))
                        nc.vector.tensor_copy(
                            out=sumT[:, soff + w * 128: soff + (w + 1) * 128],
                            in_=pw[:])

            # ---------------- phase B: tables -------------------------------
            with tc.tile_pool(name="pb", bufs=2) as pb, \
                 tc.tile_pool(name="pbs", bufs=1, space="PSUM") as pbs:

                # grand totals (transposed): [64, 4] cols = t0, t1, t2
                totL = pp.tile([U, 4], dt.float32)
                nc.vector.memset(totL[:], 0.0)
                nc.vector.tensor_reduce(
                    out=totL[:, 0:1], in_=sumT[:, off_c0:off_c0 + MP],
                    axis=mybir.AxisListType.X, op=mybir.AluOpType.add)
                nc.vector.tensor_reduce(
                    out=totL[:, 1:2], in_=sumT[:, off_c1:off_c1 + MP],
                    axis=mybir.AxisListType.X, op=mybir.AluOpType.add)
                nc.vector.tensor_reduce(
                    out=totL[:, 2:3], in_=sumT[:, off_r2:off_r2 + NP],
                    axis=mybir.AxisListType.X, op=mybir.AluOpType.add)

                totb = dram.tile([U, 4], dt.float32)
                totg = dram.tile([U, 4], dt.float32, addr_space="Shared")
                nc.gpsimd.dma_start(out=totb[:], in_=totL[:])
                nc.gpsimd.collective_compute(
                    "AllReduce", mybir.AluOpType.add,
                    ins=[totb.opt()], outs=[totg.opt()],
                    replica_groups=[list(range(NCORES))])
                totG = pp.tile([U, 4], dt.float32)
                nc.gpsimd.dma_start(out=totG[:], in_=totg[:])

                mv = pp.tile([U, 4], dt.float32)
                for j, nnz in ((0, meta["NNZ0"]), (1, meta["NNZ1"]),
                               (2, meta["NNZ2"])):
                    nc.vector.tensor_scalar_mul(
                        out=mv[:, j:j + 1], in0=totG[:, j:j + 1],
                        scalar1=float(1.0 / nnz))
                gp = pbs.tile([U, 1], dt.float32, space="PSUM", tag="gp")
                nc.tensor.matmul(gp[:], lhsT=ths["th11"][:], rhs=mv[:, 0:1],
                                 start=True, stop=False)
                nc.tensor.matmul(gp[:], lhsT=ths["th1x0_11"][:], rhs=mv[:, 1:2],
                                 start=False, stop=False)
                nc.tensor.matmul(gp[:], lhsT=ths["th2x0_11"][:], rhs=mv[:, 2:3],
                                 start=False, stop=True)
                g_t = pp.tile([U, 1], dt.float32)
                nc.vector.tensor_add(out=g_t[:], in0=gp[:], in1=thb_t[:])

                invs = {}
                for nm, t, ln in (("inv_c0", inv_c0, MP), ("inv_r0", inv_r0, NP),
                                  ("inv_c1", inv_c1, MP), ("inv_r2", inv_r2, NP)):
                    invs[nm] = pab.tile([1, ln], dt.float32, name=nm + "_t")
                    nc.sync.dma_start(out=invs[nm][:], in_=t.ap())

                ctrt_slice = dram.tile([TBL, U], dt.float32)
                ctrt_all = dram.tile([NCORES * TBL, U], dt.float32,
                                     addr_space="Shared")

                ct_stage = pp.tile([128, NWc, U], dt.float32)
                rt_stage = pp.tile([128, NWr, U], dt.float32)

                def table_chunk(ci, inv_a, inv_b, soff_a, soff_b, thA, thB,
                                add_g, stage):
                    sl = slice(ci * 128, (ci + 1) * 128)
                    pr = pbs.tile([U, 128], dt.float32, space="PSUM", tag="pr")
                    nc.tensor.matmul(pr[:], lhsT=ones_t[:], rhs=inv_a[:, sl],
                                     start=True, stop=True)
                    m0 = pb.tile([U, 128], dt.float32, tag="m0")
                    nc.vector.tensor_mul(out=m0[:],
                                         in0=sumT[:, soff_a + ci * 128:
                                                  soff_a + (ci + 1) * 128],
                                         in1=pr[:])
                    pr2 = pbs.tile([U, 128], dt.float32, space="PSUM", tag="pr2")
                    nc.tensor.matmul(pr2[:], lhsT=ones_t[:], rhs=inv_b[:, sl],
                                     start=True, stop=True)
                    m1 = pb.tile([U, 128], dt.float32, tag="m1")
                    nc.vector.tensor_mul(out=m1[:],
                                         in0=sumT[:, soff_b + ci * 128:
                                                  soff_b + (ci + 1) * 128],
                                         in1=pr2[:])
                    pc = pbs.tile([U, 128], dt.float32, space="PSUM", tag="pc")
                    nc.tensor.matmul(pc[:], lhsT=thA[:], rhs=m0[:],
                                     start=True, stop=False)
                    nc.tensor.matmul(pc[:], lhsT=thB[:], rhs=m1[:],
                                     start=False, stop=True)
                    cf = pb.tile([U, 128], dt.float32, tag="cf")
                    if add_g:
                        nc.vector.tensor_tensor(
                            out=cf[:], in0=pc[:],
                            in1=g_t[:].to_broadcast([U, 128]),
                            op=mybir.AluOpType.add)
                    else:
                        nc.vector.tensor_copy(out=cf[:], in_=pc[:])
                    pt = pbs.tile([128, U], dt.float32, space="PSUM", tag="pt")
                    nc.tensor.transpose(out=pt[:], in_=cf[:],
                                        identity=ident_t[:U, :U])
                    nc.vector.tensor_copy(out=stage[:, ci, :], in_=pt[:])

                for ci in range(NWc):
                    table_chunk(ci, invs["inv_c0"], invs["inv_c1"], off_c0,
                                off_c1, ths["th10"], ths["th1x0_10"], True,
                                ct_stage)
                for ci in range(NWr):
                    table_chunk(ci, invs["inv_r0"], invs["inv_r2"], off_r0,
                                off_r2, ths["th01"], ths["th2x0_01"], False,
                                rt_stage)

                nc.sync.dma_start(
                    out=ctrt_slice[0:MP].rearrange("(c p) f -> p c f", p=128),
                    in_=ct_stage[:])
                nc.sync.dma_start(
                    out=ctrt_slice[MP:TBL].rearrange("(c p) f -> p c f", p=128),
                    in_=rt_stage[:])
                nc.gpsimd.collective_compute(
                    "AllGather", mybir.AluOpType.bypass,
                    ins=[ctrt_slice.opt()], outs=[ctrt_all.opt()],
                    replica_groups=[list(range(NCORES))])
                if debug:
                    nc.sync.dma_start(out=sum_dump.ap(), in_=sumT[:])
                    with tc.tile_pool(name="pdbg", bufs=2) as pdbg:
                        for b in range(NCORES * TBL // 128):
                            dtile = pdbg.tile([128, U], dt.float32, tag="dt")
                            nc.sync.dma_start(
                                out=dtile[:],
                                in_=ctrt_all[b * 128:(b + 1) * 128])
                            nc.sync.dma_start(
                                out=tbl_dump.ap()[b * 128:(b + 1) * 128],
                                in_=dtile[:])

            pab_cm.__exit__(None, None, None)

            # ---------------- phase C: per-entry output ---------------------
            with tc.tile_pool(name="pc1", bufs=2) as pc1, \
                 tc.tile_pool(name="pct", bufs=4) as pct, \
                 tc.tile_pool(name="pcs", bufs=6, space="PSUM") as pcs, \
                 tc.tile_pool(name="ppos", bufs=1) as ppos:

                cpos_t = ppos.tile([128, (S0c // 64) * 520], dt.int16)
                nc.sync.dma_start(out=cpos_t[:], in_=cpos.ap())
                rpos_t = ppos.tile([128, (S0c // 64) * 520], dt.int16)
                nc.sync.dma_start(out=rpos_t[:], in_=rpos.ap())

                bias_rows = 32767 if NCORES * TBL > 32767 else 0
                gather_src = ctrt_all[bias_rows:]

                NMAC = S0c // 64
                for m in range(NMAC):
                    xw2 = pc1.tile([128, 32 * 128], dt.bfloat16, tag="xw2")
                    nc.sync.dma_start(
                        out=xw2[:],
                        in_=xT2.ap()[:, m * 4096:(m + 1) * 4096])
                    ctg = pc1.tile([128, 65, U], dt.float32, tag="ctg")
                    nc.gpsimd.dma_gather(
                        out_ap=ctg[:], in_ap=gather_src,
                        idxs_ap=cpos_t[:, m * 520:(m + 1) * 520],
                        num_idxs=65 * 128, num_idxs_reg=65 * 128, elem_size=U,
                        single_packet=False, queue_num=(2 * m) % 4)
                    rtg = pc1.tile([128, 65, U], dt.float32, tag="rtg")
                    nc.gpsimd.dma_gather(
                        out_ap=rtg[:], in_ap=gather_src,
                        idxs_ap=rpos_t[:, m * 520:(m + 1) * 520],
                        num_idxs=65 * 128, num_idxs_reg=65 * 128, elem_size=U,
                        single_packet=False, queue_num=(2 * m + 1) % 4)
                    if debug and m == 0:
                        nc.sync.dma_start(out=ctg_dump.ap(), in_=ctg[:])
                    ost = pc1.tile([128, 64, U], dt.float32, tag="ost")
                    for t in range(64):
                        q, j = t % 2, t // 2
                        py = pcs.tile([128, U], dt.float32, space="PSUM", tag="py")
                        nc.tensor.matmul(
                            py[:],
                            lhsT=xw2[64 * q:64 * (q + 1), j * 128:(j + 1) * 128],
                            rhs=th00_t[64 * q:64 * (q + 1), :],
                            start=True, stop=True)
                        if debug and m == 0:
                            yd = pct.tile([128, U], dt.float32, tag="yd")
                            nc.vector.tensor_copy(out=yd[:], in_=py[:])
                            nc.sync.dma_start(out=y0_dump.ap()[t], in_=yd[:])
                        t1 = pct.tile([128, U], dt.float32, tag="t1")
                        nc.vector.tensor_tensor(out=t1[:], in0=py[:],
                                                in1=ctg[:, t, :],
                                                op=mybir.AluOpType.add)
                        t2 = pct.tile([128, U], dt.float32, tag="t2")
                        nc.vector.tensor_tensor(out=t2[:], in0=t1[:],
                                                in1=rtg[:, t, :],
                                                op=mybir.AluOpType.add)
                        nc.scalar.activation(
                            out=ost[:, t, :], in_=t2[:],
                            func=mybir.ActivationFunctionType.Relu)
                    nc.sync.dma_start(
                        out=out_d.ap()[m * 64:(m + 1) * 64].rearrange(
                            "s p f -> p s f"),
                        in_=ost[:])

    nc.compile()
    _PROG_CACHE[key] = nc
    return nc


# --------------------------------------------------------------------------
# entry point
# --------------------------------------------------------------------------

def _run(inputs, dims, trace=False, debug=False):
    meta, in_maps, post = _prepare(inputs, dims)
    nc = _build_program(meta, debug=debug)
    res = run_bass_kernel_spmd(nc, in_maps, core_ids=list(range(NCORES)),
                               trace=trace)
    NNZ0 = dims["NNZ0"]
    out = np.empty((NNZ0, U), np.float32)
    for c in range(NCORES):
        o = res.results[c]["out_d"].reshape(-1, 128, U)
        p = post[c]
        if p["clen"] == 0:
            continue
        rows = o[p["first_slot"]].reshape(-1, U)[:p["clen"]]
        out[p["corder"]] = rows
    return out, res


def kernel(**inputs):
    out, _ = _run(inputs, FULL_DIMS, trace=False)
    return out


# ------- helpers for test harness ------------------------------------------

def install_ntff_hook():
    """Enable NTFF profiling under axon (exec_time_ns in results)."""
    try:
        import antenv
        import contextlib as _cl
        mod = types.ModuleType("antenv.axon_hooks")
        _h = [None]
        mod.set_axon_ntff_profile_hook = lambda h: _h.__setitem__(0, h)
        mod.get_axon_ntff_profile_hook = lambda: _h[0]
        sys.modules["antenv.axon_hooks"] = mod
        antenv.axon_hooks = mod
        from trn_agent_boot.trn_boot import _ntff_profile_via_ctypes
        mod.set_axon_ntff_profile_hook(
            _ntff_profile_via_ctypes("/opt/axon/libaxon_pjrt.so"))
        return True
    except Exception as e:  # pragma: no cover
        print("ntff hook install failed:", e)
        return False


def ref_numpy(inputs, dims):
    """Numpy port of the reference (for arbitrary dims)."""
    N, M = dims["N"], dims["M"]
    x0 = np.asarray(inputs["t0_values"], np.float64)
    x1 = np.asarray(inputs["t1_values"], np.float64)
    x2 = np.asarray(inputs["t2_values"], np.float64)
    tr = np.asarray(inputs["t0_rows"]); tcl = np.asarray(inputs["t0_cols"])
    t1c = np.asarray(inputs["t1_cols"]); t2r = np.asarray(inputs["t2_rows"])

    def segmean(v, ids, n):
        s = np.zeros((n, v.shape[1])); np.add.at(s, ids, v)
        c = np.bincount(ids, minlength=n).astype(np.float64)
        return s / (c + EPS)[:, None]

    th = {k: np.asarray(inputs[k], np.float64) for k in
          ("theta_00", "theta_10", "theta_01", "theta_11", "theta_1x0_10",
           "theta_1x0_11", "theta_2x0_01", "theta_2x0_11")}
    vals = x0 @ th["theta_00"]
    vals += (segmean(x0, tcl, M) @ th["theta_10"])[tcl]
    vals += (segmean(x0, tr, N) @ th["theta_01"])[tr]
    vals += x0.mean(0) @ th["theta_11"]
    vals += (segmean(x1, t1c, M) @ th["theta_1x0_10"])[tcl]
    vals += x1.mean(0) @ th["theta_1x0_11"]
    vals += (segmean(x2, t2r, N) @ th["theta_2x0_01"])[tr]
    vals += x2.mean(0) @ th["theta_2x0_11"]
    vals += np.asarray(inputs["theta_b"], np.float64)
    return np.maximum(vals, 0.0).astype(np.float32)



# revision 4
# speedup vs baseline: 2.8668x; 1.0259x over previous
"""Trainium2 Bass kernel for nn_ExchangeableLayer (segment_reduce) — v2.

out[e] = relu( x0[e] @ th00 + CT[col_e] + RT[row_e] + g )
  CT[m] = segmean(t0,cols)[m] @ th10 + segmean(t1,t1cols)[m] @ th1x0_10 + g
  RT[n] = segmean(t0,rows)[n] @ th01 + segmean(t2,t2rows)[n] @ th2x0_01
  g     = mean terms + theta_b  (folded into CT)

Design (per core, col-range sharded for t0/t1 col segs, row-range for rows):
  A) windowed one-hot segment sums on PE (sumT [64, segs]); one-hots built
     on DVE 4 slots per op; x streamed via partition-major contiguous DMA.
  B) scale by 1/(cnt+eps), apply thetas, transpose to row-major tables:
     ct_stage [128, NWc, 64] bf16 kept in SBUF (local cols only);
     rt slice [NP, 128] bf16 -> AllGather -> rt_all [8*NP, 128] global.
  C) per-entry output in transposed orientation psum[64, 512] per chunk:
     x-term (lhsT=th00), ct-term (lhsT=ct_stage window, rhs=one-hot from
     DRAM), rt rows fetched with dma_gather spread across 4 SWDGE queues
     and accumulated via transpose-matmuls; relu on ACT reads PSUM.
"""

import math
import os
import sys
import types

import numpy as np

for _p in ("/root/.axon_site/_ro/trn_rl_repo", "/opt/trn_rl_repo"):
    if os.path.isdir(_p) and _p not in sys.path:
        sys.path.append(_p)

import ml_dtypes

import concourse.bass as bass
import concourse.mybir as mybir
from concourse import bacc, tile
from concourse.bass_utils import run_bass_kernel_spmd

BF16 = ml_dtypes.bfloat16
F32 = np.float32
NCORES = 8
U = 64
WIN = 128
CHK = 512            # phase-C chunk (psum width)
MACC = 16            # chunks per macro
QCH = 4              # chunks per gather call
SENT = 128           # sentinel idxs appended per gather call
EPS = 1e-10

FULL_DIMS = dict(N=50000, M=10000, NNZ0=1_000_000, NNZ1=500_000, NNZ2=500_000)


# --------------------------------------------------------------------------
# host-side preparation
# --------------------------------------------------------------------------

def _stream(ids, seg_sl, tpad):
    """Sort entries by id; per core pad each 128-seg window to mult of tpad.

    Returns dict with per-core 'perm' (orig entry idx, -1 = pad) and 'rel'
    (window-relative seg id, -1 = pad), plus uniform per-window tile counts.
    """
    order = np.argsort(ids, kind="stable").astype(np.int64)
    sids = ids[order]
    bounds = np.searchsorted(sids, seg_sl * np.arange(NCORES + 1))
    NW = -(-seg_sl // WIN)
    # per-core per-window counts
    cnt = np.zeros((NCORES, NW), np.int64)
    locs, perms = [], []
    for c in range(NCORES):
        lo, hi = int(bounds[c]), int(bounds[c + 1])
        loc = (sids[lo:hi] - seg_sl * c).astype(np.int64)
        ws = np.searchsorted(loc, WIN * np.arange(NW + 1))
        cnt[c] = np.diff(ws)
        locs.append(loc)
        perms.append(order[lo:hi])
    kw = -(-cnt.max(axis=0) // tpad)          # uniform tiles(=tpad units)/window
    kw = np.maximum(kw, 1)
    wlen = kw * tpad                           # entries per window (padded)
    starts = np.concatenate([[0], np.cumsum(wlen)])
    E = int(starts[-1])
    cores = []
    for c in range(NCORES):
        perm = np.full(E, -1, np.int64)
        rel = np.full(E, -1, np.int64)
        ws = np.searchsorted(locs[c], WIN * np.arange(NW + 1))
        for w in range(NW):
            a, b = int(ws[w]), int(ws[w + 1])
            s0 = int(starts[w])
            perm[s0:s0 + b - a] = perms[c][a:b]
            rel[s0:s0 + b - a] = locs[c][a:b] - WIN * w
        cores.append(dict(perm=perm, rel=rel))
    return dict(NW=NW, kw=kw, wlen=wlen, starts=starts, E=E, cores=cores)


def _xa_rel(x_ext, perm, rel):
    """Phase-A arrays: xa [128, T, 64] bf16 partition-major; rel [128, T]."""
    E = len(perm)
    T = E // 128
    xa = x_ext[perm].reshape(T, 128, U).transpose(1, 0, 2)
    rl = rel.reshape(T, 128).T.astype(np.float32)
    return (np.ascontiguousarray(xa).astype(BF16),
            np.ascontiguousarray(rl).astype(BF16))


def _wrap_idx(lst):
    """Flat int16 idx list -> [128, ceil(n/16)] wrapped + 8x replicated."""
    n = len(lst)
    nf = -(-n // 16)
    pad = np.zeros(nf * 16, np.int16)
    pad[:n] = lst
    w = pad.reshape(nf, 16).T
    return np.concatenate([w] * 8, axis=0)


def _prepare(inputs, dims):
    N, M = dims["N"], dims["M"]
    NNZ0, NNZ1, NNZ2 = dims["NNZ0"], dims["NNZ1"], dims["NNZ2"]
    M_SL, N_SL = M // NCORES, N // NCORES

    t0_rows = np.asarray(inputs["t0_rows"], np.int64)
    t0_cols = np.asarray(inputs["t0_cols"], np.int64)
    t1_cols = np.asarray(inputs["t1_cols"], np.int64)
    t2_rows = np.asarray(inputs["t2_rows"], np.int64)

    st0c = _stream(t0_cols, M_SL, CHK)       # t0c padded to 512 (phase C)
    st0r = _stream(t0_rows, N_SL, WIN)
    st1c = _stream(t1_cols, M_SL, WIN)
    st2r = _stream(t2_rows, N_SL, WIN)

    NWc, NWr = st0c["NW"], st0r["NW"]
    MP, NP = NWc * WIN, NWr * WIN
    NPG = NCORES * NP                        # global padded row count
    BIAS = 32767 if NPG > 32767 else 0
    E = st0c["E"]                            # phase-C entries per core
    NCHUNK = E // CHK
    NMAC = -(-NCHUNK // MACC)

    x0 = np.asarray(inputs["t0_values"], np.float32)
    x1 = np.asarray(inputs["t1_values"], np.float32)
    x2 = np.asarray(inputs["t2_values"], np.float32)
    x0e = np.concatenate([x0, np.zeros((1, U), np.float32)]).astype(BF16)
    x1e = np.concatenate([x1, np.zeros((1, U), np.float32)]).astype(BF16)
    x2e = np.concatenate([x2, np.zeros((1, U), np.float32)]).astype(BF16)

    def _inv(ids, nseg):
        cnt = np.bincount(ids, minlength=nseg).astype(np.float32)
        return (1.0 / (cnt + np.float32(EPS))).astype(np.float32)

    inv_c0, inv_r0 = _inv(t0_cols, M), _inv(t0_rows, N)
    inv_c1, inv_r2 = _inv(t1_cols, M), _inv(t2_rows, N)

    def _slice_pad(arr, sl, pad_to):
        out = np.ones(pad_to, np.float32)
        out[: sl.stop - sl.start] = arr[sl]
        return out

    rext = np.concatenate([t0_rows, [0]])    # padded-entry row -> 0
    iota_b = np.broadcast_to(np.arange(128, dtype=np.float32),
                             (128, 128)).astype(BF16)
    ident_b = np.eye(128, dtype=np.float32).astype(BF16)
    ident_f = np.eye(128, dtype=np.float32)
    ones_f = np.ones((1, U), np.float32)
    th = {k: np.asarray(inputs[k], np.float32) for k in
          ("theta_00", "theta_10", "theta_01", "theta_11", "theta_1x0_10",
           "theta_1x0_11", "theta_2x0_01", "theta_2x0_11")}
    thbT = np.asarray(inputs["theta_b"], np.float32).reshape(U, 1)

    # gather call plan (uniform across cores): per macro, calls of <=QCH chunks
    calls = []
    for m in range(NMAC):
        c0 = m * MACC
        ncm = min(MACC, NCHUNK - c0)
        q = 0
        while q * QCH < ncm:
            nch = min(QCH, ncm - q * QCH)
            calls.append((m, c0 + q * QCH, nch, q))
            q += 1
    NIW = sum((-(-(nch * CHK + SENT) // 16)) for _, _, nch, _ in calls)

    in_maps, post = [], []
    for c in range(NCORES):
        c0c, c0r = st0c["cores"][c], st0r["cores"][c]
        c1c, c2r = st1c["cores"][c], st2r["cores"][c]

        xa0c, rel0c = _xa_rel(x0e, c0c["perm"], c0c["rel"])
        xa0r, rel0r = _xa_rel(x0e, c0r["perm"], c0r["rel"])
        xa1c, rel1c = _xa_rel(x1e, c1c["perm"], c1c["rel"])
        xa2r, rel2r = _xa_rel(x2e, c2r["perm"], c2r["rel"])

        perm = c0c["perm"]
        xcT = np.ascontiguousarray(x0e[perm].T)                 # [64, E] bf16
        ohc = (c0c["rel"][None, :] ==
               np.arange(128)[:, None]).astype(BF16)            # [128, E]

        rows = rext[perm]                                        # pad -> row 0
        rp = (rows // N_SL) * NP + (rows % N_SL)                 # padded global
        ridx = (rp - BIAS).astype(np.int64)
        iw = np.zeros((128, NIW), np.int16)
        off = 0
        for _, ch0, nch, _ in calls:
            nreal = nch * CHK
            lst = np.zeros(nreal + SENT, np.int64)
            lst[:nreal] = ridx[ch0 * CHK: ch0 * CHK + nreal]
            w = _wrap_idx(lst.astype(np.int16))
            iw[:, off:off + w.shape[1]] = w
            off += w.shape[1]

        m = dict(
            xa0c=xa0c, rel0c=rel0c, xa0r=xa0r, rel0r=rel0r,
            xa1c=xa1c, rel1c=rel1c, xa2r=xa2r, rel2r=rel2r,
            xcT=xcT, ohc=ohc, ridx_w=iw,
            inv_c0=_slice_pad(inv_c0, slice(c * M_SL, (c + 1) * M_SL),
                              MP).reshape(1, MP),
            inv_r0=_slice_pad(inv_r0, slice(c * N_SL, (c + 1) * N_SL),
                              NP).reshape(1, NP),
            inv_c1=_slice_pad(inv_c1, slice(c * M_SL, (c + 1) * M_SL),
                              MP).reshape(1, MP),
            inv_r2=_slice_pad(inv_r2, slice(c * N_SL, (c + 1) * N_SL),
                              NP).reshape(1, NP),
            iota_b=iota_b, ident_b=ident_b, ident_f=ident_f, ones_f=ones_f,
            th10=th["theta_10"], th1x0_10=th["theta_1x0_10"],
            th01=th["theta_01"], th2x0_01=th["theta_2x0_01"],
            th11=th["theta_11"], th1x0_11=th["theta_1x0_11"],
            th2x0_11=th["theta_2x0_11"],
            th00_b=th["theta_00"].astype(BF16),
            thbT=thbT,
        )
        in_maps.append(m)
        post.append(perm)

    meta = dict(
        T0c=st0c["E"] // 128, T0r=st0r["E"] // 128,
        T1c=st1c["E"] // 128, T2r=st2r["E"] // 128,
        kw0c=tuple(int(k) * (CHK // WIN) for k in st0c["kw"]),
        kw0r=tuple(int(k) for k in st0r["kw"]),
        kw1c=tuple(int(k) for k in st1c["kw"]),
        kw2r=tuple(int(k) for k in st2r["kw"]),
        NWc=NWc, NWr=NWr, MP=MP, NP=NP, NPG=NPG, BIAS=BIAS,
        E=E, NCHUNK=NCHUNK, NMAC=NMAC, NIW=NIW,
        calls=tuple(calls),
        kwc_c=tuple(int(k) for k in st0c["kw"]),   # chunks per window
        NNZ0=NNZ0, NNZ1=NNZ1, NNZ2=NNZ2,
    )
    return meta, in_maps, post


# --------------------------------------------------------------------------
# device program
# --------------------------------------------------------------------------

_PROG_CACHE = {}


def _build_program(meta):
    key = str(sorted((k, v) for k, v in meta.items() if k != "calls"))
    if key in _PROG_CACHE:
        return _PROG_CACHE[key]

    T0c, T0r, T1c, T2r = meta["T0c"], meta["T0r"], meta["T1c"], meta["T2r"]
    NWc, NWr = meta["NWc"], meta["NWr"]
    MP, NP, NPG, BIAS = meta["MP"], meta["NP"], meta["NPG"], meta["BIAS"]
    E, NCHUNK, NMAC, NIW = meta["E"], meta["NCHUNK"], meta["NMAC"], meta["NIW"]
    calls = meta["calls"]
    kwc_c = meta["kwc_c"]
    dt = mybir.dt

    nc = bacc.Bacc("TRN2", target_bir_lowering=False, debug=False,
                   num_devices=NCORES, num_swdge_queues=4)

    def din(name, shape, dty):
        return nc.dram_tensor(name, list(shape), dty, kind="ExternalInput")

    xa0c = din("xa0c", [128, T0c, U], dt.bfloat16)
    xa0r = din("xa0r", [128, T0r, U], dt.bfloat16)
    xa1c = din("xa1c", [128, T1c, U], dt.bfloat16)
    xa2r = din("xa2r", [128, T2r, U], dt.bfloat16)
    rel0c = din("rel0c", [128, T0c], dt.bfloat16)
    rel0r = din("rel0r", [128, T0r], dt.bfloat16)
    rel1c = din("rel1c", [128, T1c], dt.bfloat16)
    rel2r = din("rel2r", [128, T2r], dt.bfloat16)
    xcT = din("xcT", [U, E], dt.bfloat16)
    ohc = din("ohc", [128, E], dt.bfloat16)
    ridx_w = din("ridx_w", [128, NIW], dt.int16)
    inv_c0 = din("inv_c0", [1, MP], dt.float32)
    inv_r0 = din("inv_r0", [1, NP], dt.float32)
    inv_c1 = din("inv_c1", [1, MP], dt.float32)
    inv_r2 = din("inv_r2", [1, NP], dt.float32)
    iota_b = din("iota_b", [128, 128], dt.bfloat16)
    ident_b = din("ident_b", [128, 128], dt.bfloat16)
    ident_f = din("ident_f", [128, 128], dt.float32)
    ones_f = din("ones_f", [1, U], dt.float32)
    th10 = din("th10", [U, U], dt.float32)
    th1x0_10 = din("th1x0_10", [U, U], dt.float32)
    th01 = din("th01", [U, U], dt.float32)
    th2x0_01 = din("th2x0_01", [U, U], dt.float32)
    th11 = din("th11", [U, U], dt.float32)
    th1x0_11 = din("th1x0_11", [U, U], dt.float32)
    th2x0_11 = din("th2x0_11", [U, U], dt.float32)
    th00_b = din("th00_b", [U, U], dt.bfloat16)
    thbT = din("thbT", [U, 1], dt.float32)

    out_d = nc.dram_tensor("out_d", [U, E], dt.float32, kind="ExternalOutput")

    TOT = 2 * (MP + NP)
    off_c0, off_r0, off_c1, off_r2 = 0, MP, MP + NP, MP + NP + MP
    MAC = 32                                  # phase-A tiles per DMA macro
    K4 = 4                                    # one-hot slots per DVE op

    with tile.TileContext(nc) as tc:
        import contextlib
        with contextlib.ExitStack() as ctx:
            pp = ctx.enter_context(tc.tile_pool(name="persist", bufs=1))
            dram = ctx.enter_context(tc.tile_pool(name="dram", bufs=1,
                                                  space="DRAM"))

            pab_cm = tc.tile_pool(name="pab", bufs=1)
            pab = pab_cm.__enter__()
            sumT = pab.tile([U, TOT], dt.float32)

            iota_t = pp.tile([128, 128], dt.bfloat16)
            nc.sync.dma_start(out=iota_t[:], in_=iota_b.ap())
            identb_t = pp.tile([128, 128], dt.bfloat16)
            nc.sync.dma_start(out=identb_t[:], in_=ident_b.ap())
            identf_t = pp.tile([128, 128], dt.float32)
            nc.sync.dma_start(out=identf_t[:], in_=ident_f.ap())
            ones_t = pp.tile([1, U], dt.float32)
            nc.sync.dma_start(out=ones_t[:], in_=ones_f.ap())
            ths = {}
            for nm, t in (("th10", th10), ("th1x0_10", th1x0_10),
                          ("th01", th01), ("th2x0_01", th2x0_01),
                          ("th11", th11), ("th1x0_11", th1x0_11),
                          ("th2x0_11", th2x0_11)):
                ths[nm] = pp.tile([U, U], dt.float32, name=nm + "_t")
                nc.sync.dma_start(out=ths[nm][:], in_=t.ap())
            th00_t = pp.tile([U, U], dt.bfloat16)
            nc.sync.dma_start(out=th00_t[:], in_=th00_b.ap())
            thb_t = pp.tile([U, 1], dt.float32)
            nc.sync.dma_start(out=thb_t[:], in_=thbT.ap())
            ct_stage = pp.tile([128, NWc, U], dt.bfloat16)
            ridx_t = pp.tile([128, NIW], dt.int16)
            nc.sync.dma_start(out=ridx_t[:], in_=ridx_w.ap())

            # ---------------- phase A: windowed one-hot segment sums -------
            with tc.tile_pool(name="pa", bufs=3) as pa, \
                 tc.tile_pool(name="poh", bufs=4) as poh, \
                 tc.tile_pool(name="pas", bufs=4, space="PSUM") as pas, \
                 tc.tile_pool(name="prel", bufs=1) as prel:

                streams = [
                    (xa0c, rel0c, T0c, meta["kw0c"], off_c0),
                    (xa0r, rel0r, T0r, meta["kw0r"], off_r0),
                    (xa1c, rel1c, T1c, meta["kw1c"], off_c1),
                    (xa2r, rel2r, T2r, meta["kw2r"], off_r2),
                ]
                for si, (xa, rel_d, T, kw, soff) in enumerate(streams):
                    rel_t = prel.tile([128, T], dt.bfloat16, name=f"rel_t{si}")
                    nc.sync.dma_start(out=rel_t[:], in_=rel_d.ap())
                    # window boundaries in tile units
                    wb = [0]
                    for k in kw:
                        wb.append(wb[-1] + k)
                    # macro x loads + per-slot matmuls
                    oh_tiles = {}
                    pw_live = {}
                    for m0 in range(0, T, MAC):
                        nt = min(MAC, T - m0)
                        xw = pa.tile([128, MAC, U], dt.bfloat16, tag="xw")
                        nc.sync.dma_start(
                            out=xw[:, :nt, :],
                            in_=xa.ap()[:, m0:m0 + nt, :])
                        for t0 in range(m0, m0 + nt, K4):
                            kk = min(K4, m0 + nt - t0)
                            oh = poh.tile([128, K4, 128], dt.bfloat16,
                                          tag="oh")
                            nc.vector.tensor_tensor(
                                out=oh[:, :kk, :],
                                in0=rel_t[:, t0:t0 + kk].unsqueeze(2)
                                    .to_broadcast([128, kk, 128]),
                                in1=iota_t[:].unsqueeze(1)
                                    .to_broadcast([128, kk, 128]),
                                op=mybir.AluOpType.is_equal)
                            for j in range(kk):
                                oh_tiles[t0 + j] = (oh, j)
                        # consume tiles of this macro
                        for w in range(len(kw)):
                            a = max(wb[w], m0)
                            b = min(wb[w + 1], m0 + nt)
                            if a >= b:
                                continue
                            if w not in pw_live:
                                pw_live[w] = pas.tile(
                                    [U, 128], dt.float32, space="PSUM",
                                    tag=f"pw{w % 2}", name=f"pw_{si}_{w}")
                            pw = pw_live[w]
                            for t in range(a, b):
                                oh, j = oh_tiles.pop(t)
                                nc.tensor.matmul(
                                    pw[:], lhsT=xw[:, t - m0, :],
                                    rhs=oh[:, j, :],
                                    start=(t == wb[w]),
                                    stop=(t == wb[w + 1] - 1))
                            if b == wb[w + 1]:
                                nc.vector.tensor_copy(
                                    out=sumT[:, soff + w * 128:
                                             soff + (w + 1) * 128],
                                    in_=pw[:])
                                del pw_live[w]

            # ---------------- phase B: tables ------------------------------
            rt_all = dram.tile([NPG, 128], dt.bfloat16, addr_space="Shared")
            with tc.tile_pool(name="pb", bufs=2) as pb, \
                 tc.tile_pool(name="pbs", bufs=1, space="PSUM") as pbs:

                totL = pp.tile([U, 4], dt.float32)
                nc.vector.memset(totL[:], 0.0)
                nc.vector.tensor_reduce(
                    out=totL[:, 0:1], in_=sumT[:, off_c0:off_c0 + MP],
                    axis=mybir.AxisListType.X, op=mybir.AluOpType.add)
                nc.vector.tensor_reduce(
                    out=totL[:, 1:2], in_=sumT[:, off_c1:off_c1 + MP],
                    axis=mybir.AxisListType.X, op=mybir.AluOpType.add)
                nc.vector.tensor_reduce(
                    out=totL[:, 2:3], in_=sumT[:, off_r2:off_r2 + NP],
                    axis=mybir.AxisListType.X, op=mybir.AluOpType.add)
                totb = dram.tile([U, 4], dt.float32)
                totg = dram.tile([U, 4], dt.float32, addr_space="Shared")
                nc.gpsimd.dma_start(out=totb[:], in_=totL[:])
                nc.gpsimd.collective_compute(
                    "AllReduce", mybir.AluOpType.add,
                    ins=[totb.opt()], outs=[totg.opt()],
                    replica_groups=[list(range(NCORES))])
                totG = pp.tile([U, 4], dt.float32)
                nc.gpsimd.dma_start(out=totG[:], in_=totg[:])
                mv = pp.tile([U, 4], dt.float32)
                for j, nnz in ((0, meta["NNZ0"]), (1, meta["NNZ1"]),
                               (2, meta["NNZ2"])):
                    nc.vector.tensor_scalar_mul(
                        out=mv[:, j:j + 1], in0=totG[:, j:j + 1],
                        scalar1=float(1.0 / nnz))
                gp = pbs.tile([U, 1], dt.float32, space="PSUM", tag="gp")
                nc.tensor.matmul(gp[:], lhsT=ths["th11"][:], rhs=mv[:, 0:1],
                                 start=True, stop=False)
                nc.tensor.matmul(gp[:], lhsT=ths["th1x0_11"][:],
                                 rhs=mv[:, 1:2], start=False, stop=False)
                nc.tensor.matmul(gp[:], lhsT=ths["th2x0_11"][:],
                                 rhs=mv[:, 2:3], start=False, stop=True)
                g_t = pp.tile([U, 1], dt.float32)
                nc.vector.tensor_add(out=g_t[:], in0=gp[:], in1=thb_t[:])

                invs = {}
                for nm, t, ln in (("inv_c0", inv_c0, MP),
                                  ("inv_r0", inv_r0, NP),
                                  ("inv_c1", inv_c1, MP),
                                  ("inv_r2", inv_r2, NP)):
                    invs[nm] = pab.tile([1, ln], dt.float32, name=nm + "_t")
                    nc.sync.dma_start(out=invs[nm][:], in_=t.ap())

                rt_bf = pab.tile([128, NWr, 128], dt.bfloat16, name="rt_bf")
                nc.vector.memset(rt_bf[:], 0.0)
                rt_slice = dram.tile([NP, 128], dt.bfloat16)

                def table_chunk(ci, inv_a, inv_b, soff_a, soff_b, thA, thB,
                                add_g, stage_ap):
                    sl = slice(ci * 128, (ci + 1) * 128)
                    pr = pbs.tile([U, 128], dt.float32, space="PSUM",
                                  tag="pr")
                    nc.tensor.matmul(pr[:], lhsT=ones_t[:], rhs=inv_a[:, sl],
                                     start=True, stop=True)
                    m0 = pb.tile([U, 128], dt.float32, tag="m0")
                    nc.vector.tensor_mul(
                        out=m0[:], in0=sumT[:, soff_a + ci * 128:
                                            soff_a + (ci + 1) * 128],
                        in1=pr[:])
                    pr2 = pbs.tile([U, 128], dt.float32, space="PSUM",
                                   tag="pr2")
                    nc.tensor.matmul(pr2[:], lhsT=ones_t[:], rhs=inv_b[:, sl],
                                     start=True, stop=True)
                    m1 = pb.tile([U, 128], dt.float32, tag="m1")
                    nc.vector.tensor_mul(
                        out=m1[:], in0=sumT[:, soff_b + ci * 128:
                                            soff_b + (ci + 1) * 128],
                        in1=pr2[:])
                    pc = pbs.tile([U, 128], dt.float32, space="PSUM",
                                  tag="pc")
                    nc.tensor.matmul(pc[:], lhsT=thA[:], rhs=m0[:],
                                     start=True, stop=False)
                    nc.tensor.matmul(pc[:], lhsT=thB[:], rhs=m1[:],
                                     start=False, stop=True)
                    cf = pb.tile([U, 128], dt.float32, tag="cf")
                    if add_g:
                        nc.vector.tensor_tensor(
                            out=cf[:], in0=pc[:],
                            in1=g_t[:].to_broadcast([U, 128]),
                            op=mybir.AluOpType.add)
                    else:
                        nc.vector.tensor_copy(out=cf[:], in_=pc[:])
                    pt = pbs.tile([128, U], dt.float32, space="PSUM",
                                  tag="pt")
                    nc.tensor.transpose(out=pt[:], in_=cf[:],
                                        identity=identf_t[:U, :U])
                    nc.vector.tensor_copy(out=stage_ap(ci), in_=pt[:])

                for ci in range(NWc):
                    table_chunk(ci, invs["inv_c0"], invs["inv_c1"], off_c0,
                                off_c1, ths["th10"], ths["th1x0_10"], True,
                                lambda ci: ct_stage[:, ci, :])
                for ci in range(NWr):
                    table_chunk(ci, invs["inv_r0"], invs["inv_r2"], off_r0,
                                off_r2, ths["th01"], ths["th2x0_01"], False,
                                lambda ci: rt_bf[:, ci, 0:U])

                nc.sync.dma_start(
                    out=rt_slice[:].rearrange("(w p) f -> p w f", p=128),
                    in_=rt_bf[:])
                nc.gpsimd.collective_compute(
                    "AllGather", mybir.AluOpType.bypass,
                    ins=[rt_slice.opt()], outs=[rt_all.opt()],
                    replica_groups=[list(range(NCORES))])

            pab_cm.__exit__(None, None, None)

            # ---------------- phase C: per-entry output --------------------
            gather_src = rt_all[BIAS:]
            # chunk -> window map
            cw = []
            for w, k in enumerate(kwc_c):
                cw += [w] * k
            with tc.tile_pool(name="pcx", bufs=2) as pcx, \
                 tc.tile_pool(name="pco", bufs=2) as pco, \
                 tc.tile_pool(name="pcg", bufs=2) as pcg, \
                 tc.tile_pool(name="pcs", bufs=4, space="PSUM") as pcs, \
                 tc.tile_pool(name="pst", bufs=2) as pst:

                ioff = 0
                for mi, (m, ch0, nch, q) in enumerate(calls):
                    pass  # offsets precomputed below

                # idx col offsets per call
                ioffs = []
                off = 0
                for _, _, nch, _ in calls:
                    ioffs.append(off)
                    off += -(-(nch * CHK + SENT) // 16)

                call_of_chunk = {}
                for cidx, (m, ch0, nch, q) in enumerate(calls):
                    for jj in range(nch):
                        call_of_chunk[ch0 + jj] = (cidx, jj)

                for m in range(NMAC):
                    ch_lo = m * MACC
                    ch_hi = min(ch_lo + MACC, NCHUNK)
                    ne = (ch_hi - ch_lo) * CHK
                    e0 = ch_lo * CHK
                    xct = pcx.tile([U, MACC * CHK], dt.bfloat16, tag="xct")
                    nc.sync.dma_start(out=xct[:, :ne],
                                      in_=xcT.ap()[:, e0:e0 + ne])
                    oht = pco.tile([128, MACC * CHK], dt.bfloat16, tag="oht")
                    nc.sync.dma_start(out=oht[:, :ne],
                                      in_=ohc.ap()[:, e0:e0 + ne])
                    # gathers for this macro
                    gt = {}
                    for cidx, (mm, ch0, nch, q) in enumerate(calls):
                        if mm != m:
                            continue
                        nidx = nch * CHK + SENT
                        g = pcg.tile([128, (QCH * CHK + SENT) // 128, 128],
                                     dt.bfloat16, tag=f"g{q}",
                                     name=f"g_{m}_{q}")
                        nc.gpsimd.dma_gather(
                            out_ap=g[:, :nidx // 128, :], in_ap=gather_src,
                            idxs_ap=ridx_t[:, ioffs[cidx]:
                                           ioffs[cidx] + nidx // 16],
                            num_idxs=nidx, num_idxs_reg=nidx,
                            elem_size=128, single_packet=False,
                            queue_num=q)
                        gt[ch0] = g
                    ost = pst.tile([U, MACC * CHK], dt.float32, tag="ost")
                    for ch in range(ch_lo, ch_hi):
                        j = ch - ch_lo
                        w = cw[ch]
                        ps = pcs.tile([U, CHK], dt.float32, space="PSUM",
                                      tag="ps")
                        nc.tensor.matmul(
                            ps[:], lhsT=th00_t[:],
                            rhs=xct[:, j * CHK:(j + 1) * CHK],
                            start=True, stop=False)
                        cidx, jj = call_of_chunk[ch]
                        gch0 = calls[cidx][1]
                        g = gt[gch0]
                        for t in range(CHK // 128):
                            st = (ch - gch0) * (CHK // 128) + t
                            nc.tensor.matmul(
                                ps[:, t * 128:(t + 1) * 128],
                                lhsT=g[:, st, 0:U], rhs=identb_t[:],
                                start=False, stop=False,
                                skip_group_check=True)
                        nc.tensor.matmul(
                            ps[:], lhsT=ct_stage[:, w, :],
                            rhs=oht[:, j * CHK:(j + 1) * CHK],
                            start=False, stop=True, skip_group_check=True)
                        nc.scalar.activation(
                            out=ost[:, j * CHK:(j + 1) * CHK], in_=ps[:],
                            func=mybir.ActivationFunctionType.Relu)
                    nc.sync.dma_start(out=out_d.ap()[:, e0:e0 + ne],
                                      in_=ost[:, :ne])

    nc.compile()
    _PROG_CACHE[key] = nc
    return nc


# --------------------------------------------------------------------------
# entry point
# --------------------------------------------------------------------------

def _run(inputs, dims, trace=False):
    meta, in_maps, post = _prepare(inputs, dims)
    nc = _build_program(meta)
    res = run_bass_kernel_spmd(nc, in_maps, core_ids=list(range(NCORES)),
                               trace=trace)
    NNZ0 = dims["NNZ0"]
    out = np.empty((NNZ0, U), np.float32)
    for c in range(NCORES):
        o = res.results[c]["out_d"]          # [64, E]
        perm = post[c]
        sel = perm >= 0
        out[perm[sel]] = o.T[sel]
    return out, res


def kernel(**inputs):
    out, _ = _run(inputs, FULL_DIMS, trace=False)
    return out


# ------- helpers for test harness ------------------------------------------

def install_ntff_hook():
    try:
        import antenv
        mod = types.ModuleType("antenv.axon_hooks")
        _h = [None]
        mod.set_axon_ntff_profile_hook = lambda h: _h.__setitem__(0, h)
        mod.get_axon_ntff_profile_hook = lambda: _h[0]
        sys.modules["antenv.axon_hooks"] = mod
        antenv.axon_hooks = mod
        from trn_agent_boot.trn_boot import _ntff_profile_via_ctypes
        mod.set_axon_ntff_profile_hook(
            _ntff_profile_via_ctypes("/opt/axon/libaxon_pjrt.so"))
        return True
    except Exception as e:  # pragma: no cover
        print("ntff hook install failed:", e)
        return False


def ref_numpy(inputs, dims):
    N, M = dims["N"], dims["M"]
    x0 = np.asarray(inputs["t0_values"], np.float64)
    x1 = np.asarray(inputs["t1_values"], np.float64)
    x2 = np.asarray(inputs["t2_values"], np.float64)
    tr = np.asarray(inputs["t0_rows"]); tcl = np.asarray(inputs["t0_cols"])
    t1c = np.asarray(inputs["t1_cols"]); t2r = np.asarray(inputs["t2_rows"])

    def segmean(v, ids, n):
        s = np.zeros((n, v.shape[1])); np.add.at(s, ids, v)
        c = np.bincount(ids, minlength=n).astype(np.float64)
        return s / (c + EPS)[:, None]

    th = {k: np.asarray(inputs[k], np.float64) for k in
          ("theta_00", "theta_10", "theta_01", "theta_11", "theta_1x0_10",
           "theta_1x0_11", "theta_2x0_01", "theta_2x0_11")}
    vals = x0 @ th["theta_00"]
    vals += (segmean(x0, tcl, M) @ th["theta_10"])[tcl]
    vals += (segmean(x0, tr, N) @ th["theta_01"])[tr]
    vals += x0.mean(0) @ th["theta_11"]
    vals += (segmean(x1, t1c, M) @ th["theta_1x0_10"])[tcl]
    vals += x1.mean(0) @ th["theta_1x0_11"]
    vals += (segmean(x2, t2r, N) @ th["theta_2x0_01"])[tr]
    vals += x2.mean(0) @ th["theta_2x0_11"]
    vals += np.asarray(inputs["theta_b"], np.float64)
    return np.maximum(vals, 0.0).astype(np.float32)


if __name__ == "__main__":
    # small-scale self-test
    dims = dict(N=5000, M=1000, NNZ0=100_000, NNZ1=50_000, NNZ2=50_000)
    rng = np.random.default_rng(0)
    inputs = dict(
        t0_values=rng.standard_normal((dims["NNZ0"], U)).astype(np.float32),
        t1_values=rng.standard_normal((dims["NNZ1"], U)).astype(np.float32),
        t2_values=rng.standard_normal((dims["NNZ2"], U)).astype(np.float32),
        t0_rows=rng.integers(0, dims["N"], dims["NNZ0"]).astype(np.int32),
        t0_cols=rng.integers(0, dims["M"], dims["NNZ0"]).astype(np.int32),
        t1_cols=rng.integers(0, dims["M"], dims["NNZ1"]).astype(np.int32),
        t2_rows=rng.integers(0, dims["N"], dims["NNZ2"]).astype(np.int32),
        **{k: (rng.standard_normal((U, U)) * 0.01).astype(np.float32)
           for k in ("theta_00", "theta_10", "theta_01", "theta_11",
                     "theta_1x0_10", "theta_1x0_11", "theta_2x0_01",
                     "theta_2x0_11")},
        theta_b=(rng.standard_normal(U) * 0.01).astype(np.float32),
    )
    install_ntff_hook()
    got, res = _run(inputs, dims, trace=False)
    want = ref_numpy(inputs, dims)
    num = np.linalg.norm((got - want).astype(np.float64))
    den = np.linalg.norm(want.astype(np.float64))
    print("rel err:", num / den, "maxabs:", np.abs(got - want).max())
